# revision 1
# baseline (speedup 1.0000x reference)
"""Trainium2 Bass kernel for 8-head MultiHeadAttention (B=4, S=2048, D=512).

Sharding: tensor-parallel over heads -- core c owns head c. All matmul
operands are float32r (full fp32 bytes, 1 cycle/row on the PE at moving
dim >= 256; ~10x more accurate than bf16 as measured on HW). Each core:
  K^T,V^T = Wkv_h @ x^T    (packed KV projection, d-chunked matmuls; V^T is
                            PE-transposed per 512-token group into V' tiles
                            [128, 65] whose last column is ones)
  Q^T     = (Wq_h/8) @ q^T
  logits^T[k,q] = K^T.T @ Q^T per (batch, k-chunk)   [contraction Dh=64]
  expT = exp(logits^T)     (ScalarE, [128,1024] tiles; no max subtraction --
                            logits ~ N(0,1), fp32-safe)
  outT'[m,q] = sum_k V'[k,m] expT[k,q]   (row 64 accumulates sumexp free)
  y_partial = (outT / sumexp) @ Wo_h^T   (divide folded into a per-partition
                            scale of the 128-token output tiles)
Host sums the 8 partial y's and adds bo.

Software pipelining: emission order is proj(0); per batch b {attention qt0,
sums qt0, proj(b+1), attention qt1, sums qt1, y(b-1)}; y(B-1) last. The Tile
scheduler turns this into proj/y work riding under the ACT-bound attention.
PSUM: pa 2x[128,512] (proj/vtrans/y) + lp 2x[128,1024] (logits) +
op 1x[65,1024] (attnV accum) = 8 banks exactly.
Measured on HW: rel err 3.25e-4; cost-model 203.8 us/core (PE 158 busy,
DMA 146, ACT 135).
"""

import numpy as np

import concourse.bass as bass
import concourse.mybir as mybir
from concourse.tile import TileContext
from concourse.bass_utils import run_bass_kernel_spmd

# ---------------------------------------------------------------------------
# Workaround: this container's walrus rejects >1 sync wait on an InstDrain
# (TPB_CTRL). Split the TileContext exit-drain waits across single-wait NOPs.
_PATCHED = False


def _install_drain_patch():
    global _PATCHED
    if _PATCHED:
        return
    from concourse.vector_clock import ScopedClock, VectorClock

    def _split_drain_and_barrier(self, tick_clock, wait_clock):
        g = tick_clock.global_clock
        n = len(g)
        for i in range(n):
            t = g[i]
            if t > 0:
                vec = [0] * n
                vec[i] = t
                nop = self.nc.sync.nop(nofuse=True, hint=f"drain_wait_p{i}")
                wait_clock.add_sem_waits(
                    nop.ins, ScopedClock({None: VectorClock(vec)})
                )
        self.nc.sync.drain()
        self.nc.all_engine_barrier()
        assert self.sems is not None
        popped = self.nc._tile_sem_poison_stack.pop()
        assert popped is self._sem_poison
        self.nc.clear_and_free_semaphores(list(self.sems.allocated().values()))
        self.nc.all_engine_barrier()

    TileContext._drain_and_barrier = _split_drain_and_barrier
    _PATCHED = True


def _split_multi_waits(nc):
    """This walrus accepts at most ONE sync wait per instruction. Hoist extra
    waits onto same-engine NOPs inserted immediately before the instruction
    (same-engine program order preserves semantics)."""
    n_split = 0
    for blk in nc.m.functions[0].blocks:
        il = blk.instructions
        i = 0
        while i < len(il):
            inst = il[i]
            try:
                si = inst.sync_info
            except AttributeError:
                si = None
            if si is not None and si.on_wait is not None and len(si.on_wait) > 1:
                waits = list(si.on_wait)
                for j, w in enumerate(waits[:-1]):
                    nop = mybir.InstNoOp(
                        name=f"{inst.name}_hw{j}",
                        sync_info=mybir.SyncInfo(on_wait=[w], on_update=[]),
                        bass_nofuse=True,
                        engine=inst.engine,
                    )
                    il.insert(i, nop)
                    i += 1
                inst.sync_info = mybir.SyncInfo(
                    on_wait=[waits[-1]], on_update=list(si.on_update)
                )
                n_split += 1
            i += 1
    return n_split


# ---------------------------------------------------------------------------
B, S, D, H = 4, 2048, 512, 8
Dh = D // H  # 64
T = B * S  # 8192
NCORES = 8

F32 = mybir.dt.float32
F32R = mybir.dt.float32r
BF16 = mybir.dt.bfloat16
NP_BF16 = mybir.dt.np(BF16)

TT = 512  # projection token tile
NTT = T // TT  # 16
QTILE = 1024  # q tile for logits/exp
KC = 128  # k chunk (PSUM partitions)
NKT = T // KC  # 64 global k tiles
VW = Dh + 1  # V' width (ones column appended)



def _evac_bias(nc, out_ap, in_ap, bias_ap, on_act):
    """PSUM->SBUF evacuation with per-partition bias add, on ACT or DVE."""
    if on_act:
        nc.scalar.activation(
            out_ap, in_ap, mybir.ActivationFunctionType.Identity, bias=bias_ap
        )
    else:
        nc.vector.tensor_scalar_add(out_ap, in_ap, bias_ap)


def _build(reps: int = 1, loop_n: int = 0) -> bass.Bass:
    nc = bass.Bass(name="mha")
    xT = nc.dram_tensor("xT", [4, 128, T], F32R, kind="ExternalInput")
    qT = nc.dram_tensor("qT", [4, 128, T], F32R, kind="ExternalInput")
    wkv = nc.dram_tensor("wkv", [4, 128, 2 * Dh], F32R, kind="ExternalInput")
    bkv = nc.dram_tensor("bkv", [128, 1], F32, kind="ExternalInput")
    wq = nc.dram_tensor("wq", [4, 128, Dh], F32R, kind="ExternalInput")
    bq = nc.dram_tensor("bq", [Dh, 1], F32, kind="ExternalInput")
    wo = nc.dram_tensor("wo", [Dh, D], F32R, kind="ExternalInput")
    iden = nc.dram_tensor("iden", [Dh, Dh], F32R, kind="ExternalInput")
    y = nc.dram_tensor("y", [T, D], F32, kind="ExternalOutput")

    NSUB = TT // 512  # psum sub-tiles per projection token tile
    NQT = S // QTILE  # q tiles per batch
    NKC = S // KC  # k chunks per batch
    NB_ = S // 128  # 128-token tiles per batch

    with TileContext(nc) as tc:
        with (
            tc.tile_pool(name="const", bufs=1) as cpool,
            tc.tile_pool(name="persist", bufs=1) as ppool,
            tc.tile_pool(name="xin", bufs=2) as xpool,
            tc.tile_pool(name="qin", bufs=2) as qpool,
            tc.tile_pool(name="exps", bufs=4) as epool,
            tc.tile_pool(name="yout", bufs=2) as ypool,
            tc.tile_pool(name="dscr", bufs=1, space="DRAM") as dpool,
            tc.tile_pool(name="pa", bufs=2, space="PSUM") as pa,
            tc.tile_pool(name="lp", bufs=2, space="PSUM") as lp,
            tc.tile_pool(name="op", bufs=1, space="PSUM") as op,
        ):
            # ---- constants ----
            wkv_sb = cpool.tile([128, 4 * 2 * Dh], F32R)
            wq_sb = cpool.tile([128, 4 * Dh], F32R)
            wo_sb = cpool.tile([Dh, D], F32R)
            bkv_sb = cpool.tile([128, 1], F32)
            bq_sb = cpool.tile([Dh, 1], F32)
            ident_hi = cpool.tile([128, Dh], F32R)  # identity at partitions 64:128
            for c in range(4):
                nc.gpsimd.dma_start(wkv_sb[:, c * 128 : (c + 1) * 128], wkv[c])
                nc.gpsimd.dma_start(wq_sb[:, c * Dh : (c + 1) * Dh], wq[c])
            nc.gpsimd.dma_start(wo_sb[:], wo[:])
            nc.gpsimd.dma_start(bkv_sb[:], bkv[:])
            nc.gpsimd.dma_start(bq_sb[:], bq[:])
            nc.gpsimd.dma_start(ident_hi[64:128, :], iden[:])

            # ---- persistent intermediates ----
            kvt = ppool.tile([128, T], F32R)  # rows 0:64 K^T, rows 64:128 V^T
            qt = ppool.tile([Dh, T], F32R)
            vp = ppool.tile([128, VW * NKT], F32R)  # V' tiles [128, 65]
            outt = ppool.tile([VW, T], F32R)
            sums_sb = ppool.tile([128, NKT], F32R)
            recip = ppool.tile([128, NKT], F32)
            sums_dram = dpool.tile([1, T], F32R)

            for _ in range(reps):
                _lctx = tc.For_i(0, loop_n, 1) if loop_n else None
                if _lctx is not None:
                    _lctx.__enter__()
                nc.vector.memset(vp[:].bitcast(mybir.dt.uint32), 0x3F800000)

                def emit_proj(b):
                    base = b * S
                    for tt in range(S // TT):
                        t0 = base + tt * TT
                        xt_t = xpool.tile([128, 4 * TT], F32R, tag="xt")
                        qt_t = qpool.tile([128, 4 * TT], F32R, tag="qt")
                        nc.sync.dma_start(
                            xt_t[:],
                            xT[:, :, t0 : t0 + TT].rearrange("c p j -> p c j"),
                        )
                        nc.sync.dma_start(
                            qt_t[:],
                            qT[:, :, t0 : t0 + TT].rearrange("c p j -> p c j"),
                        )
                        for sub in range(NSUB):
                            s0 = t0 + sub * 512
                            o0 = sub * 512
                            kvp = pa.tile([128, 512], F32, tag="pa")
                            for c in range(4):
                                nc.tensor.matmul(
                                    kvp[:],
                                    wkv_sb[:, c * 128 : (c + 1) * 128],
                                    xt_t[:, c * TT + o0 : c * TT + o0 + 512],
                                    start=(c == 0),
                                    stop=(c == 3),
                                )
                            nc.vector.tensor_scalar_add(
                                kvt[:, s0 : s0 + 512], kvp[:], bkv_sb[:, 0:1]
                            )
                            qp = pa.tile([Dh, 512], F32, tag="pa")
                            for c in range(4):
                                nc.tensor.matmul(
                                    qp[:],
                                    wq_sb[:, c * Dh : (c + 1) * Dh],
                                    qt_t[:, c * TT + o0 : c * TT + o0 + 512],
                                    start=(c == 0),
                                    stop=(c == 3),
                                )
                            nc.vector.tensor_scalar_add(
                                qt[:, s0 : s0 + 512], qp[:], bq_sb[:, 0:1]
                            )
                            for kt in range(4):  # V' tiles for these 512 toks
                                kg = s0 // 128 + kt
                                k0 = s0 + kt * 128
                                vtp = pa.tile([128, Dh], F32R, tag="pa")
                                nc.tensor.transpose(
                                    vtp[:],
                                    kvt[64:128, k0 : k0 + 128],
                                    ident_hi[64:128, :],
                                )
                                nc.vector.tensor_copy(
                                    vp[:, kg * VW : kg * VW + Dh], vtp[:]
                                )

                def emit_attention_qt(b, qtile):
                    base = b * S
                    q0 = base + qtile * QTILE
                    po = op.tile([VW, QTILE], F32, tag="ot")
                    for kc in range(NKC):
                        kg = b * NB_ + kc
                        k0 = base + kc * KC
                        pl = lp.tile([128, QTILE], F32, tag="lt")
                        for hf in range(QTILE // 512):
                            nc.tensor.matmul(
                                pl[:, hf * 512 : (hf + 1) * 512],
                                kvt[0:64, k0 : k0 + KC],
                                qt[:, q0 + hf * 512 : q0 + (hf + 1) * 512],
                                start=True,
                                stop=True,
                            )
                        et = epool.tile([128, QTILE], F32R, tag="et")
                        nc.scalar.activation(
                            et[:], pl[:], mybir.ActivationFunctionType.Exp
                        )
                        for hf in range(QTILE // 512):
                            nc.tensor.matmul(
                                po[:, hf * 512 : (hf + 1) * 512],
                                vp[:, kg * VW : (kg + 1) * VW],
                                et[:, hf * 512 : (hf + 1) * 512],
                                start=(kc == 0),
                                stop=(kc == NKC - 1),
                            )
                    nc.vector.tensor_copy(outt[:, q0 : q0 + QTILE], po[:])

                def emit_sums_qt(b, qtile):
                    # softmax denominators for one q tile
                    base = b * S
                    q0 = base + qtile * QTILE
                    nqb = QTILE // 128
                    ft0 = q0 // 128
                    nc.sync.dma_start(
                        sums_dram[0:1, q0 : q0 + QTILE],
                        outt[Dh : Dh + 1, q0 : q0 + QTILE],
                    )
                    nc.sync.dma_start(
                        sums_sb[:, ft0 : ft0 + nqb],
                        sums_dram[0:1, q0 : q0 + QTILE].rearrange(
                            "o (f p) -> (o p) f", p=128
                        ),
                    )
                    nc.vector.reciprocal(
                        recip[:, ft0 : ft0 + nqb], sums_sb[:, ft0 : ft0 + nqb]
                    )

                def emit_y_qt(b, qtile, last=False):
                    # output projection for one q tile (2 groups of 512 tokens)
                    base = b * S
                    q0 = base + qtile * QTILE
                    nqb = QTILE // 128
                    ft0 = q0 // 128
                    for fg in range(nqb // 4):
                        g0 = q0 + fg * 512
                        yt = ypool.tile([128, 4 * 512], F32, tag="yt")
                        for j in range(4):
                            ft = ft0 + fg * 4 + j
                            f0 = ft * 128
                            py = pa.tile([128, 512], F32, tag="pa")
                            nc.tensor.matmul(
                                py[:],
                                outt[0:Dh, f0 : f0 + 128],
                                wo_sb[:],
                                start=True,
                                stop=True,
                            )
                            if last and j % 2 == 0:
                                nc.scalar.activation(
                                    yt[:, j * 512 : (j + 1) * 512],
                                    py[:],
                                    mybir.ActivationFunctionType.Copy,
                                    scale=recip[:, ft : ft + 1],
                                )
                            else:
                                nc.vector.tensor_scalar_mul(
                                    yt[:, j * 512 : (j + 1) * 512],
                                    py[:],
                                    recip[:, ft : ft + 1],
                                )
                        nc.sync.dma_start(
                            y[g0 : g0 + 512, :].rearrange("(j p) c -> p j c", p=128),
                            yt[:],
                        )

                # software-pipelined emission: attention(b) || y(b-1) || proj(b+1)
                emit_proj(0)
                for b in range(B):
                    emit_attention_qt(b, 0)
                    emit_sums_qt(b, 0)
                    if b + 1 < B:
                        emit_proj(b + 1)
                    emit_attention_qt(b, 1)
                    emit_sums_qt(b, 1)
                    if b > 0:
                        for qtile in range(NQT):
                            emit_y_qt(b - 1, qtile)
                for qtile in range(NQT):
                    emit_y_qt(B - 1, qtile, last=(qtile == NQT - 1))
                if _lctx is not None:
                    _lctx.__exit__(None, None, None)

    _split_multi_waits(nc)
    return nc


_CACHE: dict = {}


def _prep_inputs(x, q, Wq, bq, Wk, bk, Wv, bv, Wo, bo):
    x = np.asarray(x, np.float32)
    q = np.asarray(q, np.float32)
    Wq, bq = np.asarray(Wq, np.float32), np.asarray(bq, np.float32)
    Wk, bk = np.asarray(Wk, np.float32), np.asarray(bk, np.float32)
    Wv, bv = np.asarray(Wv, np.float32), np.asarray(bv, np.float32)
    Wo = np.asarray(Wo, np.float32)

    scale = 1.0 / np.sqrt(np.float32(Dh))
    xT4 = np.ascontiguousarray(x.reshape(T, D).T.reshape(4, 128, T))
    qT4 = np.ascontiguousarray(q.reshape(T, D).T.reshape(4, 128, T))
    in_maps = []
    for h in range(NCORES):
        sl = slice(h * Dh, (h + 1) * Dh)
        wkv_h = np.concatenate([Wk[sl].T, Wv[sl].T], axis=1)  # [512, 128]
        bkv_h = np.concatenate([bk[sl], bv[sl]])[:, None]  # [128, 1]
        wq_h = (Wq[sl] * scale).T  # [512, 64]
        bq_h = (bq[sl] * scale)[:, None]
        wo_h = np.ascontiguousarray(Wo[:, sl].T)  # [64, 512]
        in_maps.append(
            {
                "xT": xT4,
                "qT": qT4,
                "wkv": np.ascontiguousarray(wkv_h.reshape(4, 128, 128), np.float32),
                "bkv": np.ascontiguousarray(bkv_h, dtype=np.float32),
                "wq": np.ascontiguousarray(wq_h.reshape(4, 128, Dh), np.float32),
                "bq": np.ascontiguousarray(bq_h, dtype=np.float32),
                "wo": wo_h,
                "iden": np.eye(Dh, dtype=np.float32),
            }
        )
    return in_maps


def kernel(x, q, Wq, bq, Wk, bk, Wv, bv, Wo, bo):
    _install_drain_patch()
    if "nc" not in _CACHE:
        _CACHE["nc"] = _build()
    nc = _CACHE["nc"]
    in_maps = _prep_inputs(x, q, Wq, bq, Wk, bk, Wv, bv, Wo, bo)
    res = run_bass_kernel_spmd(nc, in_maps, core_ids=list(range(NCORES)))
    y = np.zeros((T, D), np.float64)
    for r in res.results:
        y += r["y"].astype(np.float64)
    y = (y + np.asarray(bo, np.float32).astype(np.float64)).astype(np.float32)
    return y.reshape(B, S, D)



# revision 45
# speedup vs baseline: 1.3301x; 1.3301x over previous
"""Trainium2 Bass kernel for 8-head MultiHeadAttention (B=4, S=2048, D=512).

Sharding: batch x head-group hybrid over 8 cores. Core c owns batch b=c>>1
and head-group g=c&1 (4 heads, 256 of the 512 K/V/concat dims). All matmul
operands are bf16 (1 cycle/row on the PE at ANY moving size, per the
instruction cost model; fp32r needs moving>=256). Host sums the 2 partial
y's per batch and adds the constant (bv @ Wo_own.T + bo) term, which is
exact because attention rows sum to 1.

Per core:
  K^T[dk,t] = Wk_g @ x^T     (dk=256 own dims, bias at evac, bf16)
  Q^T[dq,t] = (Wq_g*scale) @ q^T
  V[t,dv]   = x @ Wv_g^T     (computed directly in [token, dim] layout ->
                              no V transpose; bias folded to host)
  per (h, qtile of 1024, kchunk of 128):
    logits^T[k,q] = K_h^T.T @ Q_h^T          (contraction Dh=64)
    exp: ACT cols [0:S_ACT] via Exp table; DVE cols [S_ACT:1024] via a
         one-instruction Schraudolph: bf16bits = int16(logit*a + b)
    attnV: out[q,65] += exp[k, q-chunk].T @ [V_h | ones]   (stationary =
           exp chunk -> full 128x128 array use, ~2x fewer PE cycles than
           the V-stationary orientation; ldweights swaps are free)
  recip = 1/sumexp (ones column), accums evacuated scaled (per-partition
  recip) to bf16, PE-transposed to concat^T[dc,q], then
  y^T[dout,q] = Wo_g^T @ concat^T, DMA'd straight from PSUM as fp32.

PSUM: lp 2x[128,1024] (logits) + pa 2x[128,512] (proj/transpose/y) +
acc0/acc1 1x[128,260] (attnV accums incl. sumexp col) = 8 banks exactly.
"""

import numpy as np

import concourse.bass as bass
import concourse.mybir as mybir
from concourse.tile import TileContext
from concourse.bass_utils import run_bass_kernel_spmd

# ---------------------------------------------------------------------------
# Workaround: this container's walrus rejects >1 sync wait on an InstDrain
# (TPB_CTRL). Split the TileContext exit-drain waits across single-wait NOPs.
_PATCHED = False


def _install_drain_patch():
    global _PATCHED
    if _PATCHED:
        return
    from concourse.vector_clock import ScopedClock, VectorClock

    def _split_drain_and_barrier(self, tick_clock, wait_clock):
        g = tick_clock.global_clock
        n = len(g)
        for i in range(n):
            t = g[i]
            if t > 0:
                vec = [0] * n
                vec[i] = t
                nop = self.nc.sync.nop(nofuse=True, hint=f"drain_wait_p{i}")
                wait_clock.add_sem_waits(
                    nop.ins, ScopedClock({None: VectorClock(vec)})
                )
        self.nc.sync.drain()
        self.nc.all_engine_barrier()
        assert self.sems is not None
        popped = self.nc._tile_sem_poison_stack.pop()
        assert popped is self._sem_poison
        self.nc.clear_and_free_semaphores(list(self.sems.allocated().values()))
        self.nc.all_engine_barrier()

    TileContext._drain_and_barrier = _split_drain_and_barrier
    _PATCHED = True


def _split_multi_waits(nc):
    """Two fixes in one pass over the PE/engine programs:

    1. A matmult's sem waits must gate its LDWEIGHTS too: the PE pulls
       weight loads ahead of in-flight work, so a stationary operand that
       is freshly written by ACT/DVE can be loaded stale if the wait sits
       on the MATMULT only. Hoist every matmult wait onto NOPs inserted
       BEFORE its contiguous run of preceding InstLdweights.
    2. This walrus accepts at most ONE sync wait per instruction: split
       multi-waits across single-wait NOPs (same-engine program order
       preserves semantics).
    """
    n_split = 0
    for blk in nc.m.functions[0].blocks:
        il = blk.instructions
        i = 0
        while i < len(il):
            inst = il[i]
            try:
                si = inst.sync_info
            except AttributeError:
                si = None
            if si is None or not si.on_wait:
                i += 1
                continue
            waits = list(si.on_wait)
            is_mm = isinstance(inst, (mybir.InstMatmult,))
            if is_mm:
                # find start of the contiguous Ldweights run before inst
                ins_at = i
                while ins_at > 0 and isinstance(
                    il[ins_at - 1], mybir.InstLdweights
                ):
                    ins_at -= 1
            else:
                ins_at = i
            keep = None if (is_mm and ins_at < i) else waits[-1]
            move = waits if keep is None else waits[:-1]
            for j, w in enumerate(move):
                nop = mybir.InstNoOp(
                    name=f"{inst.name}_hw{j}",
                    sync_info=mybir.SyncInfo(on_wait=[w], on_update=[]),
                    bass_nofuse=True,
                    engine=inst.engine,
                )
                il.insert(ins_at, nop)
                ins_at += 1
                i += 1
            inst.sync_info = mybir.SyncInfo(
                on_wait=[] if keep is None else [keep],
                on_update=list(si.on_update),
            )
            n_split += 1
            i += 1
    return n_split


# ---------------------------------------------------------------------------
B, S, D, H = 4, 2048, 512, 8
Dh = D // H  # 64
NCORES = 8
HL = 4  # heads per core (head-group)
DK = HL * Dh  # 256 own K/V/concat dims

F32 = mybir.dt.float32
BF16 = mybir.dt.bfloat16
I16 = mybir.dt.int16
U16 = mybir.dt.uint16

QT = 1024  # q tile
NQT = S // QT  # 2
KC = 128  # k chunk
NKC = S // KC  # 16
VW = Dh + 1  # per-head V' width (ones column appended): 65
TT = 512  # projection token tile
NTT = S // TT  # 4

# exp engine split: ACT handles the A-half [0:512] of each [128,1024]
# logits tile via the Exp table; DVE handles the B-half [512:1024] via
# Schraudolph. Separate PSUM tags and separate exp tiles per half keep the
# two pipelines free of cross-engine hazards.
# Schraudolph constants: bf16bits = int16(x * 128*log2(e) + 128*(127+c)).
SCH_A = 128.0 * 1.4426950408889634
SCH_B = 128.0 * (127.0 - 0.0436) + 0.5


def _build() -> bass.Bass:
    nc = bass.Bass(name="mha2")
    xT = nc.dram_tensor("xT", [4, 128, S], BF16, kind="ExternalInput")
    qT = nc.dram_tensor("qT", [4, 128, S], BF16, kind="ExternalInput")
    wk = nc.dram_tensor("wk", [4, 128, DK], BF16, kind="ExternalInput")
    wq = nc.dram_tensor("wq", [4, 128, DK], BF16, kind="ExternalInput")
    wv = nc.dram_tensor("wv", [4, 128, DK], BF16, kind="ExternalInput")
    wo = nc.dram_tensor("wo", [2, 128, D], BF16, kind="ExternalInput")
    bk = nc.dram_tensor("bk", [2, 128, 1], F32, kind="ExternalInput")
    bq = nc.dram_tensor("bq", [2, 128, 1], F32, kind="ExternalInput")
    iden = nc.dram_tensor("iden", [128, 128], BF16, kind="ExternalInput")
    yT = nc.dram_tensor("yT", [4, 128, S], BF16, kind="ExternalOutput")

    Exp = mybir.ActivationFunctionType.Exp
    Copy = mybir.ActivationFunctionType.Copy
    Ident = mybir.ActivationFunctionType.Identity
    MUL = mybir.AluOpType.mult
    ADD = mybir.AluOpType.add

    with TileContext(nc) as tc:
        with (
            tc.tile_pool(name="const", bufs=1) as cpool,
            tc.tile_pool(name="persist", bufs=1) as ppool,
            tc.tile_pool(name="xin", bufs=1) as xpool,
            tc.tile_pool(name="exps", bufs=4) as epool,
            tc.tile_pool(name="ots", bufs=2) as opool,
            tc.tile_pool(name="diag", bufs=8) as dgpool,
            tc.tile_pool(name="pa", bufs=2, space="PSUM") as pa,
            tc.tile_pool(name="lp", bufs=2, space="PSUM") as lp,
            tc.tile_pool(name="ac", bufs=1, space="PSUM") as ac,
        ):
            # ---- constants ----
            wk_sb = cpool.tile([128, 4 * DK], BF16)
            wq_sb = cpool.tile([128, 4 * DK], BF16)
            wv_sb = cpool.tile([128, 4 * DK], BF16)
            wo_sb = cpool.tile([128, 2 * D], BF16)
            bk_sb = cpool.tile([128, 2], F32)
            bq_sb = cpool.tile([128, 2], F32)
            id_sb = cpool.tile([128, 128], BF16)
            # spread the input DMAs over four queues (SP carries x/q) so no
            # single queue serializes the startup.
            for c in range(4):
                nc.gpsimd.dma_start(wk_sb[:, c * DK : (c + 1) * DK], wk[c])
                nc.scalar.dma_start(wv_sb[:, c * DK : (c + 1) * DK], wv[c])
                nc.scalar.dma_start(wq_sb[:, c * DK : (c + 1) * DK], wq[c])
            for c in range(2):
                nc.gpsimd.dma_start(wo_sb[:, c * D : (c + 1) * D], wo[c])
                nc.gpsimd.dma_start(bk_sb[:, c : c + 1], bk[c])
                nc.gpsimd.dma_start(bq_sb[:, c : c + 1], bq[c])
            nc.gpsimd.dma_start(id_sb[:], iden[:])

            # ---- persistent intermediates ----
            kt = ppool.tile([128, 2 * S], BF16)  # K^T: dk-chunk c at cols c*S
            qt = ppool.tile([128, 2 * S], BF16)  # Q^T (scaled)
            vp = ppool.tile([128, NKC * HL * VW], BF16)  # V' per k-chunk
            ct = ppool.tile([128, 2 * S], BF16)  # concat^T (scaled)
            rc = ppool.tile([128, 64], F32)  # 1/sumexp per (h, qt2, qc)

            # ones columns of V' (never overwritten by evacs)
            ones_ap = (
                vp[:]
                .rearrange("p (t h m) -> p t h m", t=NKC, h=HL)[:, :, :, Dh : Dh + 1]
                .bitcast(U16)
            )
            nc.vector.memset(ones_ap, 0x3F80)

            # ---- projections ----
            xts, qtls = [], []
            for tg in range(NTT):
                t0 = tg * TT
                xt = xpool.tile([128, 4 * TT], BF16, tag=f"xt{tg}")
                qtl = xpool.tile([128, 4 * TT], BF16, tag=f"qt{tg}")
                nc.sync.dma_start(
                    xt[:], xT[:, :, t0 : t0 + TT].rearrange("c p j -> p c j")
                )
                nc.sync.dma_start(
                    qtl[:], qT[:, :, t0 : t0 + TT].rearrange("c p j -> p c j")
                )
                xts.append(xt)
                qtls.append(qtl)

            def emit_proj(tg):
                t0 = tg * TT
                xt, qtl = xts[tg], qtls[tg]
                for kchunk in range(2):
                    kp = pa.tile([128, TT], F32, tag="pa")
                    for c in range(4):
                        nc.tensor.matmul(
                            kp[:],
                            wk_sb[:, c * DK + kchunk * 128 : c * DK + kchunk * 128 + 128],
                            xt[:, c * TT : (c + 1) * TT],
                            start=(c == 0),
                            stop=(c == 3),
                        )
                    nc.vector.tensor_scalar_add(
                        kt[:, kchunk * S + t0 : kchunk * S + t0 + TT],
                        kp[:],
                        bk_sb[:, kchunk : kchunk + 1],
                    )
                    qp = pa.tile([128, TT], F32, tag="pa")
                    for c in range(4):
                        nc.tensor.matmul(
                            qp[:],
                            wq_sb[:, c * DK + kchunk * 128 : c * DK + kchunk * 128 + 128],
                            qtl[:, c * TT : (c + 1) * TT],
                            start=(c == 0),
                            stop=(c == 3),
                        )
                    nc.scalar.activation(
                        qt[:, kchunk * S + t0 : kchunk * S + t0 + TT],
                        qp[:],
                        Ident,
                        bias=bq_sb[:, kchunk : kchunk + 1],
                    )
                for tsub in range(4):
                    kc = tg * 4 + tsub
                    vpp = pa.tile([128, DK], F32, tag="pa")
                    for c in range(4):
                        nc.tensor.matmul(
                            vpp[:],
                            xt[:, c * TT + tsub * 128 : c * TT + tsub * 128 + 128],
                            wv_sb[:, c * DK : (c + 1) * DK],
                            start=(c == 0),
                            stop=(c == 3),
                        )
                    vdst = vp[
                        :, kc * HL * VW : (kc + 1) * HL * VW
                    ].rearrange("p (h m) -> p h m", h=HL)[:, :, 0:Dh]
                    vsrc = vpp[:].rearrange("p (h m) -> p h m", h=HL)
                    nc.vector.tensor_copy(vdst, vsrc)

            # ---- attention ----
            # carry-over transposes: emitted early in the NEXT iteration so
            # they fill the exp-warmup bubble instead of stalling the PE.
            # The "transpose" is a regular bf16 matmul out = accums.T @
            # diag(recip) which applies the softmax normalization for free
            # (diag tiles are built on the otherwise-idle Pool engine).
            pending = []  # (h, qt2, ot_tile, diag_tiles)

            def emit_pending():
                while pending:
                    h, qt2, ot, dgs = pending.pop()
                    base = (h & 1) * 64
                    cbase = (h >> 1) * S + qt2 * QT
                    for grp in range(2):
                        tr = pa.tile([128, 512], F32, tag="pa")
                        # sacrificial moving-path touch of this group's ot
                        # half (same weight-buffer pipelining trick as attnV)
                        nc.tensor.matmul(
                            tr[64:128, 0:VW],
                            id_sb[:, 0:64],
                            ot[:, grp * 272 : grp * 272 + VW],
                            start=True,
                            stop=True,
                        )
                        for j in range(4):
                            qc = grp * 4 + j
                            ob = (qc // 4) * 272 + (qc % 4) * VW
                            nc.tensor.matmul(
                                tr[0:64, j * 128 : (j + 1) * 128],
                                ot[:, ob : ob + Dh],
                                dgs[qc][:],
                                start=True,
                                stop=True,
                            )
                        dst = ct[base : base + 64, cbase + grp * 512 : cbase + grp * 512 + 512]
                        nc.scalar.activation(dst, tr[0:64, :], Copy)

            def emit_attn_iter(h, qt2):
                kbase = (h >> 1) * S
                krow = (h & 1) * 64
                q0 = qt2 * QT
                acc0 = ac.tile([128, 5 * VW], F32, tag="a0")
                acc1 = ac.tile([128, 5 * VW], F32, tag="a1")
                ets = []
                for kc in range(NKC):
                    k0 = kc * KC
                    lpa = lp.tile([128, 512], F32, tag="lpa")
                    lpb = lp.tile([128, 512], F32, tag="lpb")
                    for hf, lpt in enumerate((lpa, lpb)):
                        nc.tensor.matmul(
                            lpt[:],
                            kt[krow : krow + 64, kbase + k0 : kbase + k0 + KC],
                            qt[
                                krow : krow + 64,
                                kbase + q0 + hf * 512 : kbase + q0 + (hf + 1) * 512,
                            ],
                            start=True,
                            stop=True,
                        )
                    eta = epool.tile([128, 512], BF16, tag="ea")
                    etb = epool.tile([128, 512], BF16, tag="eb")
                    nc.scalar.activation(eta[:], lpa[:], Exp)
                    nc.vector.tensor_scalar(
                        etb[:].bitcast(I16), lpb[:], SCH_A, SCH_B, MUL, ADD
                    )
                    ets.append((eta, etb))
                    if kc == 4:
                        emit_pending()
                    if kc >= 2:
                        _attn_v(nc, ets[kc - 2], vp, acc0, acc1, h, kc - 2, id_sb)
                    yield
                _attn_v(nc, ets[NKC - 2], vp, acc0, acc1, h, NKC - 2, id_sb)
                _attn_v(nc, ets[NKC - 1], vp, acc0, acc1, h, NKC - 1, id_sb)

                # softmax denominators -> diag(recip) tiles (Pool), and
                # raw accumulator evacuation (2 wide instructions)
                # ot halves start 16B-aligned (272 = 34*16/2B) so the two
                # writer engines never share an SBUF line.
                rcol = (h * NQT + qt2) * 8
                ot = opool.tile([128, 272 + 4 * VW], BF16, tag="ot")
                sums_sb = opool.tile([128, 8], F32, tag="sums")
                for ai, accx in enumerate((acc0, acc1)):
                    ob = ai * 272
                    sview = accx[:].rearrange("p (j c) -> p j c", c=VW)[
                        :, 0:4, Dh : Dh + 1
                    ]
                    sdst = sums_sb[:, ai * 4 : ai * 4 + 4].rearrange(
                        "p (j o) -> p j o", o=1
                    )
                    nc.vector.tensor_copy(sdst, sview)
                    if ai == 0:
                        nc.scalar.activation(
                            ot[:, ob : ob + 4 * VW], accx[:, 0 : 4 * VW], Copy
                        )
                    else:
                        nc.vector.tensor_copy(
                            ot[:, ob : ob + 4 * VW], accx[:, 0 : 4 * VW]
                        )
                nc.vector.reciprocal(rc[:, rcol : rcol + 8], sums_sb[:])
                dgs = []
                for qc in range(8):
                    dg = dgpool.tile([128, 128], BF16, tag="dg")
                    nc.gpsimd.tensor_scalar_mul(
                        dg[:], id_sb[:], rc[:, rcol + qc : rcol + qc + 1]
                    )
                    dgs.append(dg)
                pending.append((h, qt2, ot, dgs))

            # ---- output projection helper ----
            def emit_y(qg):
                yt = opool.tile([128, 4 * 512], BF16, tag="yt")
                for do in range(4):
                    ypool_, ytag = ytags[(qg * 4 + do) % 3]
                    yp = ypool_.tile([128, 512], F32, tag=ytag)
                    for c in range(2):
                        nc.tensor.matmul(
                            yp[:],
                            wo_sb[:, c * D + do * 128 : c * D + do * 128 + 128],
                            ct[:, c * S + qg * 512 : c * S + (qg + 1) * 512],
                            start=(c == 0),
                            stop=(c == 1),
                        )
                    dst = yt[:, do * 512 : (do + 1) * 512]
                    if do % 2 == 0:
                        nc.scalar.activation(dst, yp[:], Copy)
                    else:
                        nc.vector.tensor_copy(dst, yp[:])
                nc.sync.dma_start(
                    yT[:, :, qg * 512 : (qg + 1) * 512].rearrange("c p j -> p c j"),
                    yt[:],
                )

            ytags = [(pa, "pa"), (lp, "lpa"), (lp, "lpb")]

            # interleave: proj(tg0,tg1) -> first-iteration kc0-7 can start
            # while proj(tg2,tg3) still runs, warming the exp pipelines.
            # Iterations run qt2-major so y(qg0,qg1) can be emitted as soon
            # as the qt2=0 block (all heads) finishes.
            emit_proj(0)
            emit_proj(1)
            it0 = emit_attn_iter(0, 0)
            for _ in range(8):
                next(it0)
            emit_proj(2)
            for _ in range(4):
                next(it0)
            emit_proj(3)
            for _ in it0:
                pass
            for qt2 in range(NQT):
                for h in range(HL):
                    if h == 0 and qt2 == 0:
                        continue
                    for _ in emit_attn_iter(h, qt2):
                        pass
                emit_pending()
                for qg in (0, 1) if qt2 == 0 else (2, 3):
                    emit_y(qg)

    _split_multi_waits(nc)
    return nc


def _attn_v(nc, et_pair, vp, acc0, acc1, h, kc, id_sb):
    eta, etb = et_pair
    # PSUM `start=True` clears has_written for the WHOLE BANK (measured on
    # HW): with several accumulation groups per bank, each group's start
    # would wipe its neighbours' flags and their first contribution would
    # be overwritten. So a single sacrificial matmul per bank carries the
    # one start=True (clearing the bank at kc==0, output discarded), and
    # every real group writes start=False: the first write lands on
    # has_written=0 and overwrites, later ones accumulate. The sacrificial
    # also streams the fresh exp tile via the MOVING operand, which keeps
    # the PE weight-buffer pipeline from loading the et stationaries
    # before the ACT/DVE writes are visible.
    nc.tensor.matmul(
        acc0[:, 4 * VW : 5 * VW],
        id_sb[:],
        eta[:, 0:VW],
        start=(kc == 0),
        stop=(kc == NKC - 1),
    )
    nc.tensor.matmul(
        acc1[:, 4 * VW : 5 * VW],
        id_sb[:],
        etb[:, 0:VW],
        start=(kc == 0),
        stop=(kc == NKC - 1),
    )
    for qc in range(8):
        accx = acc0 if qc < 4 else acc1
        etx = eta if qc < 4 else etb
        j = qc % 4
        nc.tensor.matmul(
            accx[:, j * VW : (j + 1) * VW],
            etx[:, j * 128 : (j + 1) * 128],
            vp[:, (kc * HL + h) * VW : (kc * HL + h + 1) * VW],
            start=False,
            stop=(kc == NKC - 1),
            skip_group_check=True,
        )


_CACHE: dict = {}


def _to_bf16(a):
    import ml_dtypes

    return np.asarray(a, dtype=ml_dtypes.bfloat16)


def _prep_inputs(x, q, Wq, bq, Wk, bk, Wv, bv, Wo, bo):
    x = np.asarray(x, np.float32)
    q = np.asarray(q, np.float32)
    Wq, bq = np.asarray(Wq, np.float32), np.asarray(bq, np.float32)
    Wk, bk = np.asarray(Wk, np.float32), np.asarray(bk, np.float32)
    Wv = np.asarray(Wv, np.float32)
    Wo = np.asarray(Wo, np.float32)

    scale = np.float32(1.0 / np.sqrt(np.float32(Dh)))
    iden = np.eye(128, dtype=np.float32)
    in_maps = []
    for c in range(NCORES):
        b, g = c >> 1, c & 1
        rows = slice(g * DK, (g + 1) * DK)
        xT = np.ascontiguousarray(x[b].T.reshape(4, 128, S))
        qT = np.ascontiguousarray(q[b].T.reshape(4, 128, S))
        wk_h = np.ascontiguousarray(Wk[rows].T.reshape(4, 128, DK))
        wq_h = np.ascontiguousarray((Wq[rows] * scale).T.reshape(4, 128, DK))
        wv_h = np.ascontiguousarray(Wv[rows].T.reshape(4, 128, DK))
        wo_h = np.ascontiguousarray(Wo[:, rows].T.reshape(2, 128, D))
        in_maps.append(
            {
                "xT": _to_bf16(xT),
                "qT": _to_bf16(qT),
                "wk": _to_bf16(wk_h),
                "wq": _to_bf16(wq_h),
                "wv": _to_bf16(wv_h),
                "wo": _to_bf16(wo_h),
                "bk": np.ascontiguousarray(bk[rows].reshape(2, 128, 1)),
                "bq": np.ascontiguousarray(
                    (bq[rows] * scale).reshape(2, 128, 1), dtype=np.float32
                ),
                "iden": _to_bf16(iden),
            }
        )
    return in_maps


def kernel(x, q, Wq, bq, Wk, bk, Wv, bv, Wo, bo):
    _install_drain_patch()
    if "nc" not in _CACHE:
        _CACHE["nc"] = _build()
    nc = _CACHE["nc"]
    in_maps = _prep_inputs(x, q, Wq, bq, Wk, bk, Wv, bv, Wo, bo)
    res = run_bass_kernel_spmd(nc, in_maps, core_ids=list(range(NCORES)))
    # constant term: attention rows sum to exactly 1, so the V bias
    # contributes bv @ Wo.T independent of the data; add with bo on host.
    const = (
        np.asarray(bv, np.float64) @ np.asarray(Wo, np.float64).T
        + np.asarray(bo, np.float64)
    )
    y = np.zeros((B, S, D), np.float64)
    for c in range(NCORES):
        b = c >> 1
        yt = res.results[c]["yT"].astype(np.float64)  # [4, 128, S]
        y[b] += yt.reshape(D, S).T
    y += const
    return y.astype(np.float32)


# revision 51
# speedup vs baseline: 1.3837x; 1.0402x over previous
"""Trainium2 Bass kernel for 8-head MultiHeadAttention (B=4, S=2048, D=512).

Sharding: batch x head-group hybrid over 8 cores. Core c owns batch b=c>>1
and head-group g=c&1 (4 heads, 256 of the 512 K/V/concat dims). All matmul
operands are bf16 (1 cycle/row on the PE at ANY moving size, per the
instruction cost model; fp32r needs moving>=256). Host sums the 2 partial
y's per batch and adds the constant (bv @ Wo_own.T + bo) term, which is
exact because attention rows sum to 1.

Per core:
  K^T[dk,t] = Wk_g @ x^T     (dk=256 own dims, bias at evac, bf16)
  Q^T[dq,t] = (Wq_g*scale) @ q^T
  V[t,dv]   = x @ Wv_g^T     (computed directly in [token, dim] layout ->
                              no V transpose; bias folded to host)
  per (h, qtile of 1024, kchunk of 128):
    logits^T[k,q] = K_h^T.T @ Q_h^T          (contraction Dh=64)
    exp: ACT cols [0:S_ACT] via Exp table; DVE cols [S_ACT:1024] via a
         one-instruction Schraudolph: bf16bits = int16(logit*a + b)
    attnV: out[q,65] += exp[k, q-chunk].T @ [V_h | ones]   (stationary =
           exp chunk -> full 128x128 array use, ~2x fewer PE cycles than
           the V-stationary orientation; ldweights swaps are free)
  recip = 1/sumexp (ones column), accums evacuated scaled (per-partition
  recip) to bf16, PE-transposed to concat^T[dc,q], then
  y^T[dout,q] = Wo_g^T @ concat^T, DMA'd straight from PSUM as fp32.

PSUM: lp 2x[128,1024] (logits) + pa 2x[128,512] (proj/transpose/y) +
acc0/acc1 1x[128,260] (attnV accums incl. sumexp col) = 8 banks exactly.
"""

import numpy as np

import concourse.bass as bass
import concourse.mybir as mybir
from concourse.tile import TileContext
from concourse.bass_utils import run_bass_kernel_spmd

# ---------------------------------------------------------------------------
# Workaround: this container's walrus rejects >1 sync wait on an InstDrain
# (TPB_CTRL). Split the TileContext exit-drain waits across single-wait NOPs.
_PATCHED = False


def _install_drain_patch():
    global _PATCHED
    if _PATCHED:
        return
    from concourse.vector_clock import ScopedClock, VectorClock

    def _split_drain_and_barrier(self, tick_clock, wait_clock):
        g = tick_clock.global_clock
        n = len(g)
        for i in range(n):
            t = g[i]
            if t > 0:
                vec = [0] * n
                vec[i] = t
                nop = self.nc.sync.nop(nofuse=True, hint=f"drain_wait_p{i}")
                wait_clock.add_sem_waits(
                    nop.ins, ScopedClock({None: VectorClock(vec)})
                )
        self.nc.sync.drain()
        self.nc.all_engine_barrier()
        assert self.sems is not None
        popped = self.nc._tile_sem_poison_stack.pop()
        assert popped is self._sem_poison
        self.nc.clear_and_free_semaphores(list(self.sems.allocated().values()))
        self.nc.all_engine_barrier()

    TileContext._drain_and_barrier = _split_drain_and_barrier
    _PATCHED = True


def _split_multi_waits(nc):
    """Two fixes in one pass over the PE/engine programs:

    1. A matmult's sem waits must gate its LDWEIGHTS too: the PE pulls
       weight loads ahead of in-flight work, so a stationary operand that
       is freshly written by ACT/DVE can be loaded stale if the wait sits
       on the MATMULT only. Hoist every matmult wait onto NOPs inserted
       BEFORE its contiguous run of preceding InstLdweights.
    2. This walrus accepts at most ONE sync wait per instruction: split
       multi-waits across single-wait NOPs (same-engine program order
       preserves semantics).
    """
    n_split = 0
    for blk in nc.m.functions[0].blocks:
        il = blk.instructions
        i = 0
        while i < len(il):
            inst = il[i]
            try:
                si = inst.sync_info
            except AttributeError:
                si = None
            if si is None or not si.on_wait:
                i += 1
                continue
            waits = list(si.on_wait)
            is_mm = isinstance(inst, (mybir.InstMatmult,))
            if is_mm:
                # find start of the contiguous Ldweights run before inst
                ins_at = i
                while ins_at > 0 and isinstance(
                    il[ins_at - 1], mybir.InstLdweights
                ):
                    ins_at -= 1
            else:
                ins_at = i
            keep = None if (is_mm and ins_at < i) else waits[-1]
            move = waits if keep is None else waits[:-1]
            for j, w in enumerate(move):
                nop = mybir.InstNoOp(
                    name=f"{inst.name}_hw{j}",
                    sync_info=mybir.SyncInfo(on_wait=[w], on_update=[]),
                    bass_nofuse=True,
                    engine=inst.engine,
                )
                il.insert(ins_at, nop)
                ins_at += 1
                i += 1
            inst.sync_info = mybir.SyncInfo(
                on_wait=[] if keep is None else [keep],
                on_update=list(si.on_update),
            )
            n_split += 1
            i += 1
    return n_split


# ---------------------------------------------------------------------------
B, S, D, H = 4, 2048, 512, 8
Dh = D // H  # 64
NCORES = 8
HL = 4  # heads per core (head-group)
DK = HL * Dh  # 256 own K/V/concat dims

F32 = mybir.dt.float32
BF16 = mybir.dt.bfloat16
I16 = mybir.dt.int16
U16 = mybir.dt.uint16

QT = 1024  # q tile
NQT = S // QT  # 2
KC = 128  # k chunk
NKC = S // KC  # 16
VW = Dh + 1  # per-head V' width (ones column appended): 65
TT = 512  # projection token tile
NTT = S // TT  # 4

# exp engine split: ACT handles the A-half [0:512] of each [128,1024]
# logits tile via the Exp table; DVE handles the B-half [512:1024] via
# Schraudolph. Separate PSUM tags and separate exp tiles per half keep the
# two pipelines free of cross-engine hazards.
# Schraudolph constants: bf16bits = int16(x * 128*log2(e) + 128*(127+c)).
SCH_A = 128.0 * 1.4426950408889634
SCH_B = 128.0 * (127.0 - 0.0436) + 0.5


def _build() -> bass.Bass:
    nc = bass.Bass(name="mha2")
    xT = nc.dram_tensor("xT", [4, 128, S], BF16, kind="ExternalInput")
    qT = nc.dram_tensor("qT", [4, 128, S], BF16, kind="ExternalInput")
    wk = nc.dram_tensor("wk", [4, 128, DK], BF16, kind="ExternalInput")
    wq = nc.dram_tensor("wq", [4, 128, DK], BF16, kind="ExternalInput")
    wv = nc.dram_tensor("wv", [4, 128, DK], BF16, kind="ExternalInput")
    wo = nc.dram_tensor("wo", [2, 128, D], BF16, kind="ExternalInput")
    bk = nc.dram_tensor("bk", [2, 128, 1], F32, kind="ExternalInput")
    bq = nc.dram_tensor("bq", [2, 128, 1], F32, kind="ExternalInput")
    iden = nc.dram_tensor("iden", [128, 128], BF16, kind="ExternalInput")
    yT = nc.dram_tensor("yT", [4, 128, S], BF16, kind="ExternalOutput")

    Exp = mybir.ActivationFunctionType.Exp
    Copy = mybir.ActivationFunctionType.Copy
    Ident = mybir.ActivationFunctionType.Identity
    MUL = mybir.AluOpType.mult
    ADD = mybir.AluOpType.add

    with TileContext(nc) as tc:
        with (
            tc.tile_pool(name="const", bufs=1) as cpool,
            tc.tile_pool(name="persist", bufs=1) as ppool,
            tc.tile_pool(name="xin", bufs=1) as xpool,
            tc.tile_pool(name="exps", bufs=4) as epool,
            tc.tile_pool(name="ots", bufs=2) as opool,
            tc.tile_pool(name="diag", bufs=8) as dgpool,
            tc.tile_pool(name="pa", bufs=2, space="PSUM") as pa,
            tc.tile_pool(name="lp", bufs=2, space="PSUM") as lp,
            tc.tile_pool(name="ac", bufs=1, space="PSUM") as ac,
        ):
            # ---- constants ----
            wk_sb = cpool.tile([128, 4 * DK], BF16)
            wq_sb = cpool.tile([128, 4 * DK], BF16)
            wv_sb = cpool.tile([128, 4 * DK], BF16)
            wo_sb = cpool.tile([128, 2 * D], BF16)
            bk_sb = cpool.tile([128, 2], F32)
            bq_sb = cpool.tile([128, 2], F32)
            id_sb = cpool.tile([128, 128], BF16)
            # spread the input DMAs over four queues (SP carries x/q) so no
            # single queue serializes the startup.
            for c in range(4):
                nc.gpsimd.dma_start(wk_sb[:, c * DK : (c + 1) * DK], wk[c])
                nc.scalar.dma_start(wv_sb[:, c * DK : (c + 1) * DK], wv[c])
                nc.scalar.dma_start(wq_sb[:, c * DK : (c + 1) * DK], wq[c])
            for c in range(2):
                nc.gpsimd.dma_start(wo_sb[:, c * D : (c + 1) * D], wo[c])
                nc.gpsimd.dma_start(bk_sb[:, c : c + 1], bk[c])
                nc.gpsimd.dma_start(bq_sb[:, c : c + 1], bq[c])
            nc.gpsimd.dma_start(id_sb[:], iden[:])

            # ---- persistent intermediates ----
            kt = ppool.tile([128, 2 * S], BF16)  # K^T: dk-chunk c at cols c*S
            qt = ppool.tile([128, 2 * S], BF16)  # Q^T (scaled)
            vp = ppool.tile([128, NKC * HL * VW], BF16)  # V' per k-chunk
            ct = ppool.tile([128, 2 * S], BF16)  # concat^T (scaled)
            rc = ppool.tile([128, 64], F32)  # 1/sumexp per (h, qt2, qc)

            # ones columns of V' (never overwritten by evacs)
            ones_ap = (
                vp[:]
                .rearrange("p (t h m) -> p t h m", t=NKC, h=HL)[:, :, :, Dh : Dh + 1]
                .bitcast(U16)
            )
            nc.vector.memset(ones_ap, 0x3F80)

            # ---- projections ----
            xts, qtls = [], []
            for tg in range(NTT):
                t0 = tg * TT
                xt = xpool.tile([128, 4 * TT], BF16, tag=f"xt{tg}")
                qtl = xpool.tile([128, 4 * TT], BF16, tag=f"qt{tg}")
                nc.sync.dma_start(
                    xt[:], xT[:, :, t0 : t0 + TT].rearrange("c p j -> p c j")
                )
                nc.sync.dma_start(
                    qtl[:], qT[:, :, t0 : t0 + TT].rearrange("c p j -> p c j")
                )
                xts.append(xt)
                qtls.append(qtl)

            def emit_proj(tg):
                t0 = tg * TT
                xt, qtl = xts[tg], qtls[tg]
                for kchunk in range(2):
                    kp = pa.tile([128, TT], F32, tag="pa")
                    for c in range(4):
                        nc.tensor.matmul(
                            kp[:],
                            wk_sb[:, c * DK + kchunk * 128 : c * DK + kchunk * 128 + 128],
                            xt[:, c * TT : (c + 1) * TT],
                            start=(c == 0),
                            stop=(c == 3),
                        )
                    nc.vector.tensor_scalar_add(
                        kt[:, kchunk * S + t0 : kchunk * S + t0 + TT],
                        kp[:],
                        bk_sb[:, kchunk : kchunk + 1],
                    )
                    qp = pa.tile([128, TT], F32, tag="pa")
                    for c in range(4):
                        nc.tensor.matmul(
                            qp[:],
                            wq_sb[:, c * DK + kchunk * 128 : c * DK + kchunk * 128 + 128],
                            qtl[:, c * TT : (c + 1) * TT],
                            start=(c == 0),
                            stop=(c == 3),
                        )
                    nc.scalar.activation(
                        qt[:, kchunk * S + t0 : kchunk * S + t0 + TT],
                        qp[:],
                        Ident,
                        bias=bq_sb[:, kchunk : kchunk + 1],
                    )
                for tsub in range(4):
                    kc = tg * 4 + tsub
                    vpp = pa.tile([128, DK], F32, tag="pa")
                    for c in range(4):
                        nc.tensor.matmul(
                            vpp[:],
                            xt[:, c * TT + tsub * 128 : c * TT + tsub * 128 + 128],
                            wv_sb[:, c * DK : (c + 1) * DK],
                            start=(c == 0),
                            stop=(c == 3),
                        )
                    vdst = vp[
                        :, kc * HL * VW : (kc + 1) * HL * VW
                    ].rearrange("p (h m) -> p h m", h=HL)[:, :, 0:Dh]
                    vsrc = vpp[:].rearrange("p (h m) -> p h m", h=HL)
                    nc.vector.tensor_copy(vdst, vsrc)

            # ---- attention ----
            # carry-over transposes: emitted early in the NEXT iteration so
            # they fill the exp-warmup bubble instead of stalling the PE.
            # The "transpose" is a regular bf16 matmul out = accums.T @
            # diag(recip) which applies the softmax normalization for free
            # (diag tiles are built on the otherwise-idle Pool engine).
            pending = []  # (h, qt2, ot_tile, diag_tiles)

            def emit_pending():
                while pending:
                    h, qt2, ot, dgs = pending.pop()
                    base = (h & 1) * 64
                    cbase = (h >> 1) * S + qt2 * QT
                    for grp in range(2):
                        tr = pa.tile([128, 512], F32, tag="pa")
                        # sacrificial moving-path touch of this group's ot
                        # half (same weight-buffer pipelining trick as attnV)
                        nc.tensor.matmul(
                            tr[64:128, 0:VW],
                            id_sb[:, 0:64],
                            ot[:, grp * 272 : grp * 272 + VW],
                            start=True,
                            stop=True,
                        )
                        for j in range(4):
                            qc = grp * 4 + j
                            ob = (qc // 4) * 272 + (qc % 4) * VW
                            nc.tensor.matmul(
                                tr[0:64, j * 128 : (j + 1) * 128],
                                ot[:, ob : ob + Dh],
                                dgs[qc][:],
                                start=True,
                                stop=True,
                            )
                        dst = ct[base : base + 64, cbase + grp * 512 : cbase + grp * 512 + 512]
                        nc.scalar.activation(dst, tr[0:64, :], Copy)

            def emit_attn_iter(h, qt2):
                kbase = (h >> 1) * S
                krow = (h & 1) * 64
                q0 = qt2 * QT
                acc0 = ac.tile([128, 5 * VW], F32, tag="a0")
                acc1 = ac.tile([128, 5 * VW], F32, tag="a1")
                ets = []
                for kc in range(NKC):
                    k0 = kc * KC
                    lpa = lp.tile([128, 512], F32, tag="lpa")
                    lpb = lp.tile([128, 512], F32, tag="lpb")
                    for hf, lpt in enumerate((lpa, lpb)):
                        nc.tensor.matmul(
                            lpt[:],
                            kt[krow : krow + 64, kbase + k0 : kbase + k0 + KC],
                            qt[
                                krow : krow + 64,
                                kbase + q0 + hf * 512 : kbase + q0 + (hf + 1) * 512,
                            ],
                            start=True,
                            stop=True,
                        )
                    eta = epool.tile([128, 512], BF16, tag="ea")
                    etb = epool.tile([128, 512], BF16, tag="eb")
                    nc.scalar.activation(eta[:], lpa[:], Exp)
                    nc.vector.tensor_scalar(
                        etb[:].bitcast(I16), lpb[:], SCH_A, SCH_B, MUL, ADD
                    )
                    ets.append((eta, etb))
                    if kc == 4:
                        emit_pending()
                    if kc == 8 and pending_y:
                        while pending_y:
                            emit_y(pending_y.pop(), pa_only=True)
                    if kc >= 2:
                        _attn_v(nc, ets[kc - 2], vp, acc0, acc1, h, kc - 2, id_sb)
                    yield
                _attn_v(nc, ets[NKC - 2], vp, acc0, acc1, h, NKC - 2, id_sb)
                _attn_v(nc, ets[NKC - 1], vp, acc0, acc1, h, NKC - 1, id_sb)

                # softmax denominators -> diag(recip) tiles (Pool), and
                # raw accumulator evacuation (2 wide instructions)
                # ot halves start 16B-aligned (272 = 34*16/2B) so the two
                # writer engines never share an SBUF line.
                rcol = (h * NQT + qt2) * 8
                ot = opool.tile([128, 272 + 4 * VW], BF16, tag="ot")
                for ai, accx in enumerate((acc0, acc1)):
                    ob = ai * 272
                    sview = accx[:].rearrange("p (j c) -> p j c", c=VW)[
                        :, 0:4, Dh : Dh + 1
                    ]
                    rdst = rc[
                        :, rcol + ai * 4 : rcol + ai * 4 + 4
                    ].rearrange("p (j o) -> p j o", o=1)
                    nc.vector.reciprocal(rdst, sview)
                    if ai == 0:
                        nc.scalar.activation(
                            ot[:, ob : ob + 4 * VW], accx[:, 0 : 4 * VW], Copy
                        )
                    else:
                        nc.vector.tensor_copy(
                            ot[:, ob : ob + 4 * VW], accx[:, 0 : 4 * VW]
                        )
                dgs = []
                for qc in range(8):
                    dg = dgpool.tile([128, 128], BF16, tag="dg")
                    nc.gpsimd.tensor_scalar_mul(
                        dg[:], id_sb[:], rc[:, rcol + qc : rcol + qc + 1]
                    )
                    dgs.append(dg)
                pending.append((h, qt2, ot, dgs))

            # ---- output projection helper ----
            # Deferred (mid-attention) y groups may only use the pa bank:
            # lpa/lpb are the live logits ring there.
            def emit_y(qg, pa_only=False):
                yt = opool.tile([128, 4 * 512], BF16, tag="yt")
                for do in range(4):
                    ypool_, ytag = (pa, "pa") if pa_only else ytags[(qg * 4 + do) % 3]
                    yp = ypool_.tile([128, 512], F32, tag=ytag)
                    for c in range(2):
                        nc.tensor.matmul(
                            yp[:],
                            wo_sb[:, c * D + do * 128 : c * D + do * 128 + 128],
                            ct[:, c * S + qg * 512 : c * S + (qg + 1) * 512],
                            start=(c == 0),
                            stop=(c == 1),
                        )
                    dst = yt[:, do * 512 : (do + 1) * 512]
                    if do % 2 == 0:
                        nc.scalar.activation(dst, yp[:], Copy)
                    else:
                        nc.vector.tensor_copy(dst, yp[:])
                nc.sync.dma_start(
                    yT[:, :, qg * 512 : (qg + 1) * 512].rearrange("c p j -> p c j"),
                    yt[:],
                )

            ytags = [(pa, "pa"), (lp, "lpa"), (lp, "lpb")]
            pending_y = []

            # interleave: proj(tg0,tg1) -> first-iteration kc0-7 can start
            # while proj(tg2,tg3) still runs, warming the exp pipelines.
            # Iterations run qt2-major so y(qg0,qg1) can be emitted as soon
            # as the qt2=0 block (all heads) finishes.
            emit_proj(0)
            emit_proj(1)
            it0 = emit_attn_iter(0, 0)
            for _ in range(8):
                next(it0)
            emit_proj(2)
            for _ in range(4):
                next(it0)
            emit_proj(3)
            for _ in it0:
                pass
            for qt2 in range(NQT):
                for h in range(HL):
                    if h == 0 and qt2 == 0:
                        continue
                    for _ in emit_attn_iter(h, qt2):
                        pass
                if qt2 == 0:
                    pending_y.extend([1, 0])
                else:
                    emit_pending()
                    for qg in (2, 3):
                        emit_y(qg)

    _split_multi_waits(nc)
    return nc


def _attn_v(nc, et_pair, vp, acc0, acc1, h, kc, id_sb):
    eta, etb = et_pair
    # PSUM `start=True` clears has_written for the WHOLE BANK (measured on
    # HW): with several accumulation groups per bank, each group's start
    # would wipe its neighbours' flags and their first contribution would
    # be overwritten. So a single sacrificial matmul per bank carries the
    # one start=True (clearing the bank at kc==0, output discarded), and
    # every real group writes start=False: the first write lands on
    # has_written=0 and overwrites, later ones accumulate. The sacrificial
    # also streams the fresh exp tile via the MOVING operand, which keeps
    # the PE weight-buffer pipeline from loading the et stationaries
    # before the ACT/DVE writes are visible.
    nc.tensor.matmul(
        acc0[:, 4 * VW : 5 * VW],
        id_sb[:],
        eta[:, 0:VW],
        start=(kc == 0),
        stop=(kc == NKC - 1),
    )
    nc.tensor.matmul(
        acc1[:, 4 * VW : 5 * VW],
        id_sb[:],
        etb[:, 0:VW],
        start=(kc == 0),
        stop=(kc == NKC - 1),
    )
    for qc in range(8):
        accx = acc0 if qc < 4 else acc1
        etx = eta if qc < 4 else etb
        j = qc % 4
        nc.tensor.matmul(
            accx[:, j * VW : (j + 1) * VW],
            etx[:, j * 128 : (j + 1) * 128],
            vp[:, (kc * HL + h) * VW : (kc * HL + h + 1) * VW],
            start=False,
            stop=(kc == NKC - 1),
            skip_group_check=True,
        )


_CACHE: dict = {}


def _to_bf16(a):
    import ml_dtypes

    return np.asarray(a, dtype=ml_dtypes.bfloat16)


def _prep_inputs(x, q, Wq, bq, Wk, bk, Wv, bv, Wo, bo):
    x = np.asarray(x, np.float32)
    q = np.asarray(q, np.float32)
    Wq, bq = np.asarray(Wq, np.float32), np.asarray(bq, np.float32)
    Wk, bk = np.asarray(Wk, np.float32), np.asarray(bk, np.float32)
    Wv = np.asarray(Wv, np.float32)
    Wo = np.asarray(Wo, np.float32)

    scale = np.float32(1.0 / np.sqrt(np.float32(Dh)))
    iden = np.eye(128, dtype=np.float32)
    in_maps = []
    for c in range(NCORES):
        b, g = c >> 1, c & 1
        rows = slice(g * DK, (g + 1) * DK)
        xT = np.ascontiguousarray(x[b].T.reshape(4, 128, S))
        qT = np.ascontiguousarray(q[b].T.reshape(4, 128, S))
        wk_h = np.ascontiguousarray(Wk[rows].T.reshape(4, 128, DK))
        wq_h = np.ascontiguousarray((Wq[rows] * scale).T.reshape(4, 128, DK))
        wv_h = np.ascontiguousarray(Wv[rows].T.reshape(4, 128, DK))
        wo_h = np.ascontiguousarray(Wo[:, rows].T.reshape(2, 128, D))
        in_maps.append(
            {
                "xT": _to_bf16(xT),
                "qT": _to_bf16(qT),
                "wk": _to_bf16(wk_h),
                "wq": _to_bf16(wq_h),
                "wv": _to_bf16(wv_h),
                "wo": _to_bf16(wo_h),
                "bk": np.ascontiguousarray(bk[rows].reshape(2, 128, 1)),
                "bq": np.ascontiguousarray(
                    (bq[rows] * scale).reshape(2, 128, 1), dtype=np.float32
                ),
                "iden": _to_bf16(iden),
            }
        )
    return in_maps


def kernel(x, q, Wq, bq, Wk, bk, Wv, bv, Wo, bo):
    _install_drain_patch()
    if "nc" not in _CACHE:
        _CACHE["nc"] = _build()
    nc = _CACHE["nc"]
    in_maps = _prep_inputs(x, q, Wq, bq, Wk, bk, Wv, bv, Wo, bo)
    res = run_bass_kernel_spmd(nc, in_maps, core_ids=list(range(NCORES)))
    # constant term: attention rows sum to exactly 1, so the V bias
    # contributes bv @ Wo.T independent of the data; add with bo on host.
    const = (
        np.asarray(bv, np.float64) @ np.asarray(Wo, np.float64).T
        + np.asarray(bo, np.float64)
    )
    y = np.zeros((B, S, D), np.float64)
    for c in range(NCORES):
        b = c >> 1
        yt = res.results[c]["yT"].astype(np.float64)  # [4, 128, S]
        y[b] += yt.reshape(D, S).T
    y += const
    return y.astype(np.float32)


# revision 54
# speedup vs baseline: 1.3839x; 1.0002x over previous
"""Trainium2 Bass kernel for 8-head MultiHeadAttention (B=4, S=2048, D=512).

Sharding: batch x head-group hybrid over 8 cores. Core c owns batch b=c>>1
and head-group g=c&1 (4 heads, 256 of the 512 K/V/concat dims). All matmul
operands are bf16 (1 cycle/row on the PE at ANY moving size, per the
instruction cost model; fp32r needs moving>=256). Host sums the 2 partial
y's per batch and adds the constant (bv @ Wo_own.T + bo) term, which is
exact because attention rows sum to 1.

Per core:
  K^T[dk,t] = Wk_g @ x^T     (dk=256 own dims, bias at evac, bf16)
  Q^T[dq,t] = (Wq_g*scale) @ q^T
  V[t,dv]   = x @ Wv_g^T     (computed directly in [token, dim] layout ->
                              no V transpose; bias folded to host)
  per (h, qtile of 1024, kchunk of 128):
    logits^T[k,q] = K_h^T.T @ Q_h^T          (contraction Dh=64)
    exp: ACT cols [0:S_ACT] via Exp table; DVE cols [S_ACT:1024] via a
         one-instruction Schraudolph: bf16bits = int16(logit*a + b)
    attnV: out[q,65] += exp[k, q-chunk].T @ [V_h | ones]   (stationary =
           exp chunk -> full 128x128 array use, ~2x fewer PE cycles than
           the V-stationary orientation; ldweights swaps are free)
  recip = 1/sumexp (ones column), accums evacuated scaled (per-partition
  recip) to bf16, PE-transposed to concat^T[dc,q], then
  y^T[dout,q] = Wo_g^T @ concat^T, DMA'd straight from PSUM as fp32.

PSUM: lp 2x[128,1024] (logits) + pa 2x[128,512] (proj/transpose/y) +
acc0/acc1 1x[128,260] (attnV accums incl. sumexp col) = 8 banks exactly.
"""

import numpy as np

import concourse.bass as bass
import concourse.mybir as mybir
from concourse.tile import TileContext
from concourse.bass_utils import run_bass_kernel_spmd

# ---------------------------------------------------------------------------
# Workaround: this container's walrus rejects >1 sync wait on an InstDrain
# (TPB_CTRL). Split the TileContext exit-drain waits across single-wait NOPs.
_PATCHED = False


def _install_drain_patch():
    global _PATCHED
    if _PATCHED:
        return
    from concourse.vector_clock import ScopedClock, VectorClock

    def _split_drain_and_barrier(self, tick_clock, wait_clock):
        g = tick_clock.global_clock
        n = len(g)
        for i in range(n):
            t = g[i]
            if t > 0:
                vec = [0] * n
                vec[i] = t
                nop = self.nc.sync.nop(nofuse=True, hint=f"drain_wait_p{i}")
                wait_clock.add_sem_waits(
                    nop.ins, ScopedClock({None: VectorClock(vec)})
                )
        self.nc.sync.drain()
        self.nc.all_engine_barrier()
        assert self.sems is not None
        popped = self.nc._tile_sem_poison_stack.pop()
        assert popped is self._sem_poison
        self.nc.clear_and_free_semaphores(list(self.sems.allocated().values()))
        self.nc.all_engine_barrier()

    TileContext._drain_and_barrier = _split_drain_and_barrier
    _PATCHED = True


def _split_multi_waits(nc):
    """Two fixes in one pass over the PE/engine programs:

    1. A matmult's sem waits must gate its LDWEIGHTS too: the PE pulls
       weight loads ahead of in-flight work, so a stationary operand that
       is freshly written by ACT/DVE can be loaded stale if the wait sits
       on the MATMULT only. Hoist every matmult wait onto NOPs inserted
       BEFORE its contiguous run of preceding InstLdweights.
    2. This walrus accepts at most ONE sync wait per instruction: split
       multi-waits across single-wait NOPs (same-engine program order
       preserves semantics).
    """
    n_split = 0
    for blk in nc.m.functions[0].blocks:
        il = blk.instructions
        i = 0
        while i < len(il):
            inst = il[i]
            try:
                si = inst.sync_info
            except AttributeError:
                si = None
            if si is None or not si.on_wait:
                i += 1
                continue
            waits = list(si.on_wait)
            is_mm = isinstance(inst, (mybir.InstMatmult,))
            if is_mm:
                # find start of the contiguous Ldweights run before inst
                ins_at = i
                while ins_at > 0 and isinstance(
                    il[ins_at - 1], mybir.InstLdweights
                ):
                    ins_at -= 1
            else:
                ins_at = i
            keep = None if (is_mm and ins_at < i) else waits[-1]
            move = waits if keep is None else waits[:-1]
            for j, w in enumerate(move):
                nop = mybir.InstNoOp(
                    name=f"{inst.name}_hw{j}",
                    sync_info=mybir.SyncInfo(on_wait=[w], on_update=[]),
                    bass_nofuse=True,
                    engine=inst.engine,
                )
                il.insert(ins_at, nop)
                ins_at += 1
                i += 1
            inst.sync_info = mybir.SyncInfo(
                on_wait=[] if keep is None else [keep],
                on_update=list(si.on_update),
            )
            n_split += 1
            i += 1
    return n_split


# ---------------------------------------------------------------------------
B, S, D, H = 4, 2048, 512, 8
Dh = D // H  # 64
NCORES = 8
HL = 4  # heads per core (head-group)
DK = HL * Dh  # 256 own K/V/concat dims

F32 = mybir.dt.float32
BF16 = mybir.dt.bfloat16
I16 = mybir.dt.int16
U16 = mybir.dt.uint16

QT = 1024  # q tile
NQT = S // QT  # 2
KC = 128  # k chunk
NKC = S // KC  # 16
VW = Dh + 1  # per-head V' width (ones column appended): 65
TT = 512  # projection token tile
NTT = S // TT  # 4

# exp engine split: ACT handles the A-half [0:512] of each [128,1024]
# logits tile via the Exp table; DVE handles the B-half [512:1024] via
# Schraudolph. Separate PSUM tags and separate exp tiles per half keep the
# two pipelines free of cross-engine hazards.
# Schraudolph constants: bf16bits = int16(x * 128*log2(e) + 128*(127+c)).
SCH_A = 128.0 * 1.4426950408889634
SCH_B = 128.0 * (127.0 - 0.0436) + 0.5


def _build() -> bass.Bass:
    nc = bass.Bass(name="mha2")
    xT = nc.dram_tensor("xT", [4, 128, S], BF16, kind="ExternalInput")
    qT = nc.dram_tensor("qT", [4, 128, S], BF16, kind="ExternalInput")
    wk = nc.dram_tensor("wk", [4, 128, DK], BF16, kind="ExternalInput")
    wq = nc.dram_tensor("wq", [4, 128, DK], BF16, kind="ExternalInput")
    wv = nc.dram_tensor("wv", [4, 128, DK], BF16, kind="ExternalInput")
    wo = nc.dram_tensor("wo", [2, 128, D], BF16, kind="ExternalInput")
    bk = nc.dram_tensor("bk", [2, 128, 1], F32, kind="ExternalInput")
    bq = nc.dram_tensor("bq", [2, 128, 1], F32, kind="ExternalInput")
    iden = nc.dram_tensor("iden", [128, 128], BF16, kind="ExternalInput")
    yT = nc.dram_tensor("yT", [4, 128, S], BF16, kind="ExternalOutput")

    Exp = mybir.ActivationFunctionType.Exp
    Copy = mybir.ActivationFunctionType.Copy
    Ident = mybir.ActivationFunctionType.Identity
    MUL = mybir.AluOpType.mult
    ADD = mybir.AluOpType.add

    with TileContext(nc) as tc:
        with (
            tc.tile_pool(name="const", bufs=1) as cpool,
            tc.tile_pool(name="persist", bufs=1) as ppool,
            tc.tile_pool(name="xin", bufs=1) as xpool,
            tc.tile_pool(name="exps", bufs=4) as epool,
            tc.tile_pool(name="ots", bufs=2) as opool,
            tc.tile_pool(name="diag", bufs=8) as dgpool,
            tc.tile_pool(name="pa", bufs=2, space="PSUM") as pa,
            tc.tile_pool(name="lp", bufs=2, space="PSUM") as lp,
            tc.tile_pool(name="ac", bufs=1, space="PSUM") as ac,
        ):
            # ---- constants ----
            wk_sb = cpool.tile([128, 4 * DK], BF16)
            wq_sb = cpool.tile([128, 4 * DK], BF16)
            wv_sb = cpool.tile([128, 4 * DK], BF16)
            wo_sb = cpool.tile([128, 2 * D], BF16)
            bk_sb = cpool.tile([128, 2], F32)
            bq_sb = cpool.tile([128, 2], F32)
            id_sb = cpool.tile([128, 128], BF16)
            # spread the input DMAs over four queues (SP carries x/q) so no
            # single queue serializes the startup.
            for c in range(4):
                nc.gpsimd.dma_start(wk_sb[:, c * DK : (c + 1) * DK], wk[c])
                nc.scalar.dma_start(wv_sb[:, c * DK : (c + 1) * DK], wv[c])
                nc.scalar.dma_start(wq_sb[:, c * DK : (c + 1) * DK], wq[c])
            for c in range(2):
                nc.gpsimd.dma_start(wo_sb[:, c * D : (c + 1) * D], wo[c])
                nc.gpsimd.dma_start(bk_sb[:, c : c + 1], bk[c])
                nc.gpsimd.dma_start(bq_sb[:, c : c + 1], bq[c])
            nc.gpsimd.dma_start(id_sb[:], iden[:])

            # ---- persistent intermediates ----
            kt = ppool.tile([128, 2 * S], BF16)  # K^T: dk-chunk c at cols c*S
            qt = ppool.tile([128, 2 * S], BF16)  # Q^T (scaled)
            vp = ppool.tile([128, NKC * HL * VW], BF16)  # V' per k-chunk
            ct = ppool.tile([128, 2 * S], BF16)  # concat^T (scaled)
            rc = ppool.tile([128, 64], F32)  # 1/sumexp per (h, qt2, qc)

            # ones columns of V' (never overwritten by evacs)
            ones_ap = (
                vp[:]
                .rearrange("p (t h m) -> p t h m", t=NKC, h=HL)[:, :, :, Dh : Dh + 1]
                .bitcast(U16)
            )
            nc.vector.memset(ones_ap, 0x3F80)

            # ---- projections ----
            xts, qtls = [], []
            for tg in range(NTT):
                t0 = tg * TT
                xt = xpool.tile([128, 4 * TT], BF16, tag=f"xt{tg}")
                qtl = xpool.tile([128, 4 * TT], BF16, tag=f"qt{tg}")
                nc.sync.dma_start(
                    xt[:], xT[:, :, t0 : t0 + TT].rearrange("c p j -> p c j")
                )
                nc.sync.dma_start(
                    qtl[:], qT[:, :, t0 : t0 + TT].rearrange("c p j -> p c j")
                )
                xts.append(xt)
                qtls.append(qtl)

            def emit_proj(tg):
                t0 = tg * TT
                xt, qtl = xts[tg], qtls[tg]
                for kchunk in range(2):
                    kp = pa.tile([128, TT], F32, tag="pa")
                    for c in range(4):
                        nc.tensor.matmul(
                            kp[:],
                            wk_sb[:, c * DK + kchunk * 128 : c * DK + kchunk * 128 + 128],
                            xt[:, c * TT : (c + 1) * TT],
                            start=(c == 0),
                            stop=(c == 3),
                        )
                    nc.vector.tensor_scalar_add(
                        kt[:, kchunk * S + t0 : kchunk * S + t0 + TT],
                        kp[:],
                        bk_sb[:, kchunk : kchunk + 1],
                    )
                    qp = pa.tile([128, TT], F32, tag="pa")
                    for c in range(4):
                        nc.tensor.matmul(
                            qp[:],
                            wq_sb[:, c * DK + kchunk * 128 : c * DK + kchunk * 128 + 128],
                            qtl[:, c * TT : (c + 1) * TT],
                            start=(c == 0),
                            stop=(c == 3),
                        )
                    nc.scalar.activation(
                        qt[:, kchunk * S + t0 : kchunk * S + t0 + TT],
                        qp[:],
                        Ident,
                        bias=bq_sb[:, kchunk : kchunk + 1],
                    )
                for tsub in range(4):
                    kc = tg * 4 + tsub
                    vpp = pa.tile([128, DK], F32, tag="pa")
                    for c in range(4):
                        nc.tensor.matmul(
                            vpp[:],
                            xt[:, c * TT + tsub * 128 : c * TT + tsub * 128 + 128],
                            wv_sb[:, c * DK : (c + 1) * DK],
                            start=(c == 0),
                            stop=(c == 3),
                        )
                    vdst = vp[
                        :, kc * HL * VW : (kc + 1) * HL * VW
                    ].rearrange("p (h m) -> p h m", h=HL)[:, :, 0:Dh]
                    vsrc = vpp[:].rearrange("p (h m) -> p h m", h=HL)
                    nc.vector.tensor_copy(vdst, vsrc)

            # ---- attention ----
            # carry-over transposes: emitted early in the NEXT iteration so
            # they fill the exp-warmup bubble instead of stalling the PE.
            # The "transpose" is a regular bf16 matmul out = accums.T @
            # diag(recip) which applies the softmax normalization for free
            # (diag tiles are built on the otherwise-idle Pool engine).
            pending = []  # (h, qt2, ot_tile, diag_tiles)

            def emit_pending():
                while pending:
                    h, qt2, ot, dgs = pending.pop()
                    base = (h & 1) * 64
                    cbase = (h >> 1) * S + qt2 * QT
                    for grp in range(2):
                        tr = pa.tile([128, 512], F32, tag="pa")
                        # sacrificial moving-path touch of this group's ot
                        # half (same weight-buffer pipelining trick as attnV)
                        nc.tensor.matmul(
                            tr[64:128, 0:VW],
                            id_sb[:, 0:64],
                            ot[:, grp * 272 : grp * 272 + VW],
                            start=True,
                            stop=True,
                        )
                        for j in range(4):
                            qc = grp * 4 + j
                            ob = (qc // 4) * 272 + (qc % 4) * VW
                            nc.tensor.matmul(
                                tr[0:64, j * 128 : (j + 1) * 128],
                                ot[:, ob : ob + Dh],
                                dgs[qc][:],
                                start=True,
                                stop=True,
                            )
                        dst = ct[base : base + 64, cbase + grp * 512 : cbase + grp * 512 + 512]
                        nc.scalar.activation(dst, tr[0:64, :], Copy)

            def emit_attn_iter(h, qt2):
                kbase = (h >> 1) * S
                krow = (h & 1) * 64
                q0 = qt2 * QT
                acc0 = ac.tile([128, 5 * VW], F32, tag="a0")
                acc1 = ac.tile([128, 5 * VW], F32, tag="a1")
                ets = []
                for kc in range(NKC):
                    k0 = kc * KC
                    lpa = lp.tile([128, 512], F32, tag="lpa")
                    lpb = lp.tile([128, 512], F32, tag="lpb")
                    for hf, lpt in enumerate((lpa, lpb)):
                        nc.tensor.matmul(
                            lpt[:],
                            kt[krow : krow + 64, kbase + k0 : kbase + k0 + KC],
                            qt[
                                krow : krow + 64,
                                kbase + q0 + hf * 512 : kbase + q0 + (hf + 1) * 512,
                            ],
                            start=True,
                            stop=True,
                        )
                    eta = epool.tile([128, 512], BF16, tag="ea")
                    etb = epool.tile([128, 512], BF16, tag="eb")
                    nc.scalar.activation(eta[:], lpa[:], Exp)
                    nc.vector.tensor_scalar(
                        etb[:].bitcast(I16), lpb[:], SCH_A, SCH_B, MUL, ADD
                    )
                    ets.append((eta, etb))
                    if kc == 4:
                        emit_pending()
                    if kc in (8, 12) and pending_y:
                        emit_y(pending_y.pop(), pa_only=True)
                    if kc >= 2:
                        _attn_v(nc, ets[kc - 2], vp, acc0, acc1, h, kc - 2, id_sb)
                    yield
                _attn_v(nc, ets[NKC - 2], vp, acc0, acc1, h, NKC - 2, id_sb)
                _attn_v(nc, ets[NKC - 1], vp, acc0, acc1, h, NKC - 1, id_sb)

                # softmax denominators -> diag(recip) tiles (Pool), and
                # raw accumulator evacuation (2 wide instructions)
                # ot halves start 16B-aligned (272 = 34*16/2B) so the two
                # writer engines never share an SBUF line.
                rcol = (h * NQT + qt2) * 8
                ot = opool.tile([128, 272 + 4 * VW], BF16, tag="ot")
                for ai, accx in enumerate((acc0, acc1)):
                    ob = ai * 272
                    sview = accx[:].rearrange("p (j c) -> p j c", c=VW)[
                        :, 0:4, Dh : Dh + 1
                    ]
                    rdst = rc[
                        :, rcol + ai * 4 : rcol + ai * 4 + 4
                    ].rearrange("p (j o) -> p j o", o=1)
                    nc.vector.reciprocal(rdst, sview)
                    if ai == 0:
                        nc.scalar.activation(
                            ot[:, ob : ob + 4 * VW], accx[:, 0 : 4 * VW], Copy
                        )
                    else:
                        nc.vector.tensor_copy(
                            ot[:, ob : ob + 4 * VW], accx[:, 0 : 4 * VW]
                        )
                dgs = []
                for qc in range(8):
                    dg = dgpool.tile([128, 128], BF16, tag="dg")
                    nc.gpsimd.tensor_scalar_mul(
                        dg[:], id_sb[:], rc[:, rcol + qc : rcol + qc + 1]
                    )
                    dgs.append(dg)
                pending.append((h, qt2, ot, dgs))

            # ---- output projection helper ----
            # Deferred (mid-attention) y groups may only use the pa bank:
            # lpa/lpb are the live logits ring there.
            def emit_y(qg, pa_only=False):
                yt = opool.tile([128, 4 * 512], BF16, tag="yt")
                for do in range(4):
                    ypool_, ytag = (pa, "pa") if pa_only else ytags[(qg * 4 + do) % 3]
                    yp = ypool_.tile([128, 512], F32, tag=ytag)
                    for c in range(2):
                        nc.tensor.matmul(
                            yp[:],
                            wo_sb[:, c * D + do * 128 : c * D + do * 128 + 128],
                            ct[:, c * S + qg * 512 : c * S + (qg + 1) * 512],
                            start=(c == 0),
                            stop=(c == 1),
                        )
                    dst = yt[:, do * 512 : (do + 1) * 512]
                    if do % 2 == 0:
                        nc.scalar.activation(dst, yp[:], Copy)
                    else:
                        nc.vector.tensor_copy(dst, yp[:])
                nc.sync.dma_start(
                    yT[:, :, qg * 512 : (qg + 1) * 512].rearrange("c p j -> p c j"),
                    yt[:],
                )

            ytags = [(pa, "pa"), (lp, "lpa"), (lp, "lpb")]
            pending_y = []

            # interleave: proj(tg0,tg1) -> first-iteration kc0-7 can start
            # while proj(tg2,tg3) still runs, warming the exp pipelines.
            # Iterations run qt2-major so y(qg0,qg1) can be emitted as soon
            # as the qt2=0 block (all heads) finishes.
            emit_proj(0)
            emit_proj(1)
            it0 = emit_attn_iter(0, 0)
            for _ in range(8):
                next(it0)
            emit_proj(2)
            for _ in range(4):
                next(it0)
            emit_proj(3)
            for _ in it0:
                pass
            for qt2 in range(NQT):
                for h in range(HL):
                    if h == 0 and qt2 == 0:
                        continue
                    for _ in emit_attn_iter(h, qt2):
                        pass
                if qt2 == 0:
                    pending_y.extend([1, 0])
                else:
                    emit_pending()
                    for qg in (2, 3):
                        emit_y(qg)

    _split_multi_waits(nc)
    return nc


def _attn_v(nc, et_pair, vp, acc0, acc1, h, kc, id_sb):
    eta, etb = et_pair
    # PSUM `start=True` clears has_written for the WHOLE BANK (measured on
    # HW): with several accumulation groups per bank, each group's start
    # would wipe its neighbours' flags and their first contribution would
    # be overwritten. So a single sacrificial matmul per bank carries the
    # one start=True (clearing the bank at kc==0, output discarded), and
    # every real group writes start=False: the first write lands on
    # has_written=0 and overwrites, later ones accumulate. The sacrificial
    # also streams the fresh exp tile via the MOVING operand, which keeps
    # the PE weight-buffer pipeline from loading the et stationaries
    # before the ACT/DVE writes are visible.
    nc.tensor.matmul(
        acc0[:, 4 * VW : 5 * VW],
        id_sb[:],
        eta[:, 0:VW],
        start=(kc == 0),
        stop=(kc == NKC - 1),
    )
    nc.tensor.matmul(
        acc1[:, 4 * VW : 5 * VW],
        id_sb[:],
        etb[:, 0:VW],
        start=(kc == 0),
        stop=(kc == NKC - 1),
    )
    for qc in range(8):
        accx = acc0 if qc < 4 else acc1
        etx = eta if qc < 4 else etb
        j = qc % 4
        nc.tensor.matmul(
            accx[:, j * VW : (j + 1) * VW],
            etx[:, j * 128 : (j + 1) * 128],
            vp[:, (kc * HL + h) * VW : (kc * HL + h + 1) * VW],
            start=False,
            stop=(kc == NKC - 1),
            skip_group_check=True,
        )


_CACHE: dict = {}


def _to_bf16(a):
    import ml_dtypes

    return np.asarray(a, dtype=ml_dtypes.bfloat16)


def _prep_inputs(x, q, Wq, bq, Wk, bk, Wv, bv, Wo, bo):
    x = np.asarray(x, np.float32)
    q = np.asarray(q, np.float32)
    Wq, bq = np.asarray(Wq, np.float32), np.asarray(bq, np.float32)
    Wk, bk = np.asarray(Wk, np.float32), np.asarray(bk, np.float32)
    Wv = np.asarray(Wv, np.float32)
    Wo = np.asarray(Wo, np.float32)

    scale = np.float32(1.0 / np.sqrt(np.float32(Dh)))
    iden = np.eye(128, dtype=np.float32)
    in_maps = []
    for c in range(NCORES):
        b, g = c >> 1, c & 1
        rows = slice(g * DK, (g + 1) * DK)
        xT = np.ascontiguousarray(x[b].T.reshape(4, 128, S))
        qT = np.ascontiguousarray(q[b].T.reshape(4, 128, S))
        wk_h = np.ascontiguousarray(Wk[rows].T.reshape(4, 128, DK))
        wq_h = np.ascontiguousarray((Wq[rows] * scale).T.reshape(4, 128, DK))
        wv_h = np.ascontiguousarray(Wv[rows].T.reshape(4, 128, DK))
        wo_h = np.ascontiguousarray(Wo[:, rows].T.reshape(2, 128, D))
        in_maps.append(
            {
                "xT": _to_bf16(xT),
                "qT": _to_bf16(qT),
                "wk": _to_bf16(wk_h),
                "wq": _to_bf16(wq_h),
                "wv": _to_bf16(wv_h),
                "wo": _to_bf16(wo_h),
                "bk": np.ascontiguousarray(bk[rows].reshape(2, 128, 1)),
                "bq": np.ascontiguousarray(
                    (bq[rows] * scale).reshape(2, 128, 1), dtype=np.float32
                ),
                "iden": _to_bf16(iden),
            }
        )
    return in_maps


def kernel(x, q, Wq, bq, Wk, bk, Wv, bv, Wo, bo):
    _install_drain_patch()
    if "nc" not in _CACHE:
        _CACHE["nc"] = _build()
    nc = _CACHE["nc"]
    in_maps = _prep_inputs(x, q, Wq, bq, Wk, bk, Wv, bv, Wo, bo)
    res = run_bass_kernel_spmd(nc, in_maps, core_ids=list(range(NCORES)))
    # constant term: attention rows sum to exactly 1, so the V bias
    # contributes bv @ Wo.T independent of the data; add with bo on host.
    const = (
        np.asarray(bv, np.float64) @ np.asarray(Wo, np.float64).T
        + np.asarray(bo, np.float64)
    )
    y = np.zeros((B, S, D), np.float64)
    for c in range(NCORES):
        b = c >> 1
        yt = res.results[c]["yT"].astype(np.float64)  # [4, 128, S]
        y[b] += yt.reshape(D, S).T
    y += const
    return y.astype(np.float32)


# revision 60
# speedup vs baseline: 1.3842x; 1.0002x over previous
"""Trainium2 Bass kernel for 8-head MultiHeadAttention (B=4, S=2048, D=512).

Sharding: batch x head-group hybrid over 8 cores. Core c owns batch b=c>>1
and head-group g=c&1 (4 heads, 256 of the 512 K/V/concat dims). All matmul
operands are bf16 (1 cycle/row on the PE at ANY moving size; fp32r needs
moving>=256). Host sums the 2 partial y's per batch and adds the constant
(bv @ Wo.T + bo) term, which is exact because attention rows sum to 1.

Per core:
  K^T[dk,t] = Wk_g @ x^T      (dk=256 own dims, bias at evac, bf16)
  Q^T[dq,t] = (Wq_g*scale) @ q^T
  V[t,dv]   = x @ Wv_g^T      (computed directly in [token, dim] layout ->
                               no V transpose; V bias folded to host)
  per (h, qtile of 1024, kchunk of 128): two decoupled half-pipelines:
    logits^T[k, q-half] into separate 1-bank psum tiles lpa/lpb
    exp: ACT does the A half via the Exp table; DVE does the B half with a
         one-instruction Schraudolph (bf16bits = int16(logit*a + b), ~2%
         per-weight err that largely normalizes out; measured output
         rel err 8.9e-3)
    attnV: acc[q,65] += exp[k, q-chunk].T @ [V_h | ones]   (stationary =
           exp chunk -> full 128x128 array use, ~2x fewer PE cycles than
           the V-stationary orientation; ldweights swaps are free)
  recip = 1/sumexp (ones column); accums evacuated raw to bf16; the
  softmax normalization rides the "transpose" for free: a regular bf16
  matmul concat^T-chunk = accums.T @ diag(recip), with the diag tiles
  built on the otherwise-idle Pool engine (identity x per-partition recip);
  then y^T[dout,q] = Wo_g^T @ concat^T, evacuated bf16 and DMA'd out.

Software pipelining: attnV trails exp by 2 kchunks; the previous
iteration's transposes and the qt2=0 y-projection groups are emitted
inside the next iteration to fill its exp-warmup bubble; the first
iteration is interleaved into the projection phase.

PSUM: lpa/lpb 2x[128,512] each (logits halves) + pa 2x[128,512]
(proj/transpose/y) + acc0/acc1 1x[128,325] = 8 banks exactly.

Two TRN2 hardware quirks found on the way (both cost a day... er, hours):
 1. PSUM `start=True` clears has_written for the WHOLE BANK, not just the
    written region. With 4+ accumulation groups sharing a bank, each
    group's start wipes its neighbours' flags and their first (kc=0)
    contribution gets overwritten by kc=1 (exactly -1/16 of the softmax
    mass, 12.7%% output error). Fix: one sacrificial matmul per bank
    carries the only start=True; real groups always write start=False.
 2. A matmult whose stationary operand was freshly written by ACT/DVE can
    load stale bytes: the writer's semaphore fires slightly before the
    data is visible to the PE weight-load port, and the PE pulls
    LDWEIGHTS ahead of in-flight matmuls (bounded by its two weight
    buffers). The sacrificial matmuls double as protection: they stream
    the fresh exp tile as the MOVING operand (65 cycles), so the real
    LDWEIGHTS issue ~30-60ns after the semaphore.

Cost model (TimelineSim, the graded metric): 147266 ns/core vs 203800 ns
for the previous head-TP fp32r kernel; measured rel err 8.949e-3.
"""

import numpy as np

import concourse.bass as bass
import concourse.mybir as mybir
from concourse.tile import TileContext
from concourse.bass_utils import run_bass_kernel_spmd

# ---------------------------------------------------------------------------
# Workaround: this container's walrus rejects >1 sync wait on an InstDrain
# (TPB_CTRL). Split the TileContext exit-drain waits across single-wait NOPs.
_PATCHED = False


def _install_drain_patch():
    global _PATCHED
    if _PATCHED:
        return
    from concourse.vector_clock import ScopedClock, VectorClock

    def _split_drain_and_barrier(self, tick_clock, wait_clock):
        g = tick_clock.global_clock
        n = len(g)
        for i in range(n):
            t = g[i]
            if t > 0:
                vec = [0] * n
                vec[i] = t
                nop = self.nc.sync.nop(nofuse=True, hint=f"drain_wait_p{i}")
                wait_clock.add_sem_waits(
                    nop.ins, ScopedClock({None: VectorClock(vec)})
                )
        self.nc.sync.drain()
        self.nc.all_engine_barrier()
        assert self.sems is not None
        popped = self.nc._tile_sem_poison_stack.pop()
        assert popped is self._sem_poison
        self.nc.clear_and_free_semaphores(list(self.sems.allocated().values()))
        self.nc.all_engine_barrier()

    TileContext._drain_and_barrier = _split_drain_and_barrier
    _PATCHED = True


def _split_multi_waits(nc):
    """Two fixes in one pass over the PE/engine programs:

    1. A matmult's sem waits must gate its LDWEIGHTS too: the PE pulls
       weight loads ahead of in-flight work, so a stationary operand that
       is freshly written by ACT/DVE can be loaded stale if the wait sits
       on the MATMULT only. Hoist every matmult wait onto NOPs inserted
       BEFORE its contiguous run of preceding InstLdweights.
    2. This walrus accepts at most ONE sync wait per instruction: split
       multi-waits across single-wait NOPs (same-engine program order
       preserves semantics).
    """
    n_split = 0
    for blk in nc.m.functions[0].blocks:
        il = blk.instructions
        i = 0
        while i < len(il):
            inst = il[i]
            try:
                si = inst.sync_info
            except AttributeError:
                si = None
            if si is None or not si.on_wait:
                i += 1
                continue
            waits = list(si.on_wait)
            is_mm = isinstance(inst, (mybir.InstMatmult,))
            if is_mm:
                # find start of the contiguous Ldweights run before inst
                ins_at = i
                while ins_at > 0 and isinstance(
                    il[ins_at - 1], mybir.InstLdweights
                ):
                    ins_at -= 1
            else:
                ins_at = i
            keep = None if (is_mm and ins_at < i) else waits[-1]
            move = waits if keep is None else waits[:-1]
            for j, w in enumerate(move):
                nop = mybir.InstNoOp(
                    name=f"{inst.name}_hw{j}",
                    sync_info=mybir.SyncInfo(on_wait=[w], on_update=[]),
                    bass_nofuse=True,
                    engine=inst.engine,
                )
                il.insert(ins_at, nop)
                ins_at += 1
                i += 1
            inst.sync_info = mybir.SyncInfo(
                on_wait=[] if keep is None else [keep],
                on_update=list(si.on_update),
            )
            n_split += 1
            i += 1
    return n_split


# ---------------------------------------------------------------------------
B, S, D, H = 4, 2048, 512, 8
Dh = D // H  # 64
NCORES = 8
HL = 4  # heads per core (head-group)
DK = HL * Dh  # 256 own K/V/concat dims

F32 = mybir.dt.float32
BF16 = mybir.dt.bfloat16
I16 = mybir.dt.int16
U16 = mybir.dt.uint16

QT = 1024  # q tile
NQT = S // QT  # 2
KC = 128  # k chunk
NKC = S // KC  # 16
VW = Dh + 1  # per-head V' width (ones column appended): 65
TT = 512  # projection token tile
NTT = S // TT  # 4

# exp engine split: ACT handles the A-half [0:512] of each [128,1024]
# logits tile via the Exp table; DVE handles the B-half [512:1024] via
# Schraudolph. Separate PSUM tags and separate exp tiles per half keep the
# two pipelines free of cross-engine hazards.
# Schraudolph constants: bf16bits = int16(x * 128*log2(e) + 128*(127+c)).
SCH_A = 128.0 * 1.4426950408889634
SCH_B = 128.0 * (127.0 - 0.0436) + 0.5


def _build() -> bass.Bass:
    nc = bass.Bass(name="mha2")
    xT = nc.dram_tensor("xT", [4, 128, S], BF16, kind="ExternalInput")
    qT = nc.dram_tensor("qT", [4, 128, S], BF16, kind="ExternalInput")
    wk = nc.dram_tensor("wk", [4, 128, DK], BF16, kind="ExternalInput")
    wq = nc.dram_tensor("wq", [4, 128, DK], BF16, kind="ExternalInput")
    wv = nc.dram_tensor("wv", [4, 128, DK], BF16, kind="ExternalInput")
    wo = nc.dram_tensor("wo", [2, 128, D], BF16, kind="ExternalInput")
    bk = nc.dram_tensor("bk", [2, 128, 1], F32, kind="ExternalInput")
    bq = nc.dram_tensor("bq", [2, 128, 1], F32, kind="ExternalInput")
    iden = nc.dram_tensor("iden", [128, 128], BF16, kind="ExternalInput")
    yT = nc.dram_tensor("yT", [4, 128, S], BF16, kind="ExternalOutput")

    Exp = mybir.ActivationFunctionType.Exp
    Copy = mybir.ActivationFunctionType.Copy
    Ident = mybir.ActivationFunctionType.Identity
    MUL = mybir.AluOpType.mult
    ADD = mybir.AluOpType.add

    with TileContext(nc) as tc:
        with (
            tc.tile_pool(name="const", bufs=1) as cpool,
            tc.tile_pool(name="persist", bufs=1) as ppool,
            tc.tile_pool(name="xin", bufs=1) as xpool,
            tc.tile_pool(name="exps", bufs=6) as epool,
            tc.tile_pool(name="ots", bufs=3) as opool,
            tc.tile_pool(name="diag", bufs=16) as dgpool,
            tc.tile_pool(name="pa", bufs=2, space="PSUM") as pa,
            tc.tile_pool(name="lp", bufs=2, space="PSUM") as lp,
            tc.tile_pool(name="ac", bufs=1, space="PSUM") as ac,
        ):
            # ---- constants ----
            wk_sb = cpool.tile([128, 4 * DK], BF16)
            wq_sb = cpool.tile([128, 4 * DK], BF16)
            wv_sb = cpool.tile([128, 4 * DK], BF16)
            wo_sb = cpool.tile([128, 2 * D], BF16)
            bk_sb = cpool.tile([128, 2], F32)
            bq_sb = cpool.tile([128, 2], F32)
            id_sb = cpool.tile([128, 128], BF16)
            # spread the input DMAs over four queues (SP carries x/q) so no
            # single queue serializes the startup.
            for c in range(4):
                nc.gpsimd.dma_start(wk_sb[:, c * DK : (c + 1) * DK], wk[c])
                nc.scalar.dma_start(wv_sb[:, c * DK : (c + 1) * DK], wv[c])
                nc.scalar.dma_start(wq_sb[:, c * DK : (c + 1) * DK], wq[c])
            for c in range(2):
                nc.gpsimd.dma_start(wo_sb[:, c * D : (c + 1) * D], wo[c])
                nc.gpsimd.dma_start(bk_sb[:, c : c + 1], bk[c])
                nc.gpsimd.dma_start(bq_sb[:, c : c + 1], bq[c])
            nc.gpsimd.dma_start(id_sb[:], iden[:])

            # ---- persistent intermediates ----
            kt = ppool.tile([128, 2 * S], BF16)  # K^T: dk-chunk c at cols c*S
            qt = ppool.tile([128, 2 * S], BF16)  # Q^T (scaled)
            vp = ppool.tile([128, NKC * HL * VW], BF16)  # V' per k-chunk
            ct = ppool.tile([128, 2 * S], BF16)  # concat^T (scaled)
            rc = ppool.tile([128, 64], F32)  # 1/sumexp per (h, qt2, qc)

            # ones columns of V' (never overwritten by evacs)
            ones_ap = (
                vp[:]
                .rearrange("p (t h m) -> p t h m", t=NKC, h=HL)[:, :, :, Dh : Dh + 1]
                .bitcast(U16)
            )
            nc.vector.memset(ones_ap, 0x3F80)

            # ---- projections ----
            xts, qtls = [], []
            for tg in range(NTT):
                t0 = tg * TT
                xt = xpool.tile([128, 4 * TT], BF16, tag=f"xt{tg}")
                qtl = xpool.tile([128, 4 * TT], BF16, tag=f"qt{tg}")
                nc.sync.dma_start(
                    xt[:], xT[:, :, t0 : t0 + TT].rearrange("c p j -> p c j")
                )
                nc.sync.dma_start(
                    qtl[:], qT[:, :, t0 : t0 + TT].rearrange("c p j -> p c j")
                )
                xts.append(xt)
                qtls.append(qtl)

            def emit_proj(tg):
                t0 = tg * TT
                xt, qtl = xts[tg], qtls[tg]
                for kchunk in range(2):
                    kp = pa.tile([128, TT], F32, tag="pa")
                    for c in range(4):
                        nc.tensor.matmul(
                            kp[:],
                            wk_sb[:, c * DK + kchunk * 128 : c * DK + kchunk * 128 + 128],
                            xt[:, c * TT : (c + 1) * TT],
                            start=(c == 0),
                            stop=(c == 3),
                        )
                    nc.vector.tensor_scalar_add(
                        kt[:, kchunk * S + t0 : kchunk * S + t0 + TT],
                        kp[:],
                        bk_sb[:, kchunk : kchunk + 1],
                    )
                    qp = pa.tile([128, TT], F32, tag="pa")
                    for c in range(4):
                        nc.tensor.matmul(
                            qp[:],
                            wq_sb[:, c * DK + kchunk * 128 : c * DK + kchunk * 128 + 128],
                            qtl[:, c * TT : (c + 1) * TT],
                            start=(c == 0),
                            stop=(c == 3),
                        )
                    nc.scalar.activation(
                        qt[:, kchunk * S + t0 : kchunk * S + t0 + TT],
                        qp[:],
                        Ident,
                        bias=bq_sb[:, kchunk : kchunk + 1],
                    )
                for tsub in range(4):
                    kc = tg * 4 + tsub
                    vpp = pa.tile([128, DK], F32, tag="pa")
                    for c in range(4):
                        nc.tensor.matmul(
                            vpp[:],
                            xt[:, c * TT + tsub * 128 : c * TT + tsub * 128 + 128],
                            wv_sb[:, c * DK : (c + 1) * DK],
                            start=(c == 0),
                            stop=(c == 3),
                        )
                    vdst = vp[
                        :, kc * HL * VW : (kc + 1) * HL * VW
                    ].rearrange("p (h m) -> p h m", h=HL)[:, :, 0:Dh]
                    vsrc = vpp[:].rearrange("p (h m) -> p h m", h=HL)
                    nc.vector.tensor_copy(vdst, vsrc)

            # ---- attention ----
            # carry-over transposes: emitted early in the NEXT iteration so
            # they fill the exp-warmup bubble instead of stalling the PE.
            # The "transpose" is a regular bf16 matmul out = accums.T @
            # diag(recip) which applies the softmax normalization for free
            # (diag tiles are built on the otherwise-idle Pool engine).
            pending = []  # (h, qt2, ot_tile, diag_tiles)

            def emit_pending():
                while pending:
                    h, qt2, ot, dgs = pending.pop()
                    base = (h & 1) * 64
                    cbase = (h >> 1) * S + qt2 * QT
                    for grp in range(2):
                        tr = pa.tile([128, 512], F32, tag="pa")
                        # sacrificial moving-path touch of this group's ot
                        # half (same weight-buffer pipelining trick as attnV)
                        nc.tensor.matmul(
                            tr[64:128, 0:VW],
                            id_sb[:, 0:64],
                            ot[:, grp * 272 : grp * 272 + VW],
                            start=True,
                            stop=True,
                        )
                        for j in range(4):
                            qc = grp * 4 + j
                            ob = (qc // 4) * 272 + (qc % 4) * VW
                            nc.tensor.matmul(
                                tr[0:64, j * 128 : (j + 1) * 128],
                                ot[:, ob : ob + Dh],
                                dgs[qc][:],
                                start=True,
                                stop=True,
                            )
                        dst = ct[base : base + 64, cbase + grp * 512 : cbase + grp * 512 + 512]
                        nc.scalar.activation(dst, tr[0:64, :], Copy)

            def emit_attn_iter(h, qt2):
                kbase = (h >> 1) * S
                krow = (h & 1) * 64
                q0 = qt2 * QT
                acc0 = ac.tile([128, 5 * VW], F32, tag="a0")
                acc1 = ac.tile([128, 5 * VW], F32, tag="a1")
                ets = []
                for kc in range(NKC):
                    k0 = kc * KC
                    lpa = lp.tile([128, 512], F32, tag="lpa")
                    lpb = lp.tile([128, 512], F32, tag="lpb")
                    for hf, lpt in enumerate((lpa, lpb)):
                        nc.tensor.matmul(
                            lpt[:],
                            kt[krow : krow + 64, kbase + k0 : kbase + k0 + KC],
                            qt[
                                krow : krow + 64,
                                kbase + q0 + hf * 512 : kbase + q0 + (hf + 1) * 512,
                            ],
                            start=True,
                            stop=True,
                        )
                    eta = epool.tile([128, 512], BF16, tag="ea")
                    etb = epool.tile([128, 512], BF16, tag="eb")
                    nc.scalar.activation(eta[:], lpa[:], Exp)
                    nc.vector.tensor_scalar(
                        etb[:].bitcast(I16), lpb[:], SCH_A, SCH_B, MUL, ADD
                    )
                    ets.append((eta, etb))
                    if kc == 4:
                        emit_pending()
                    if kc in (8, 12) and pending_y:
                        emit_y(pending_y.pop(), pa_only=True)
                    if kc >= 2:
                        _attn_v(nc, ets[kc - 2], vp, acc0, acc1, h, kc - 2, id_sb)
                    yield
                for kcl in (NKC - 2, NKC - 1):
                    _attn_v(nc, ets[kcl], vp, acc0, acc1, h, kcl, id_sb)

                # softmax denominators -> diag(recip) tiles (Pool), and
                # raw accumulator evacuation (2 wide instructions)
                # ot halves start 16B-aligned (272 = 34*16/2B) so the two
                # writer engines never share an SBUF line.
                rcol = (h * NQT + qt2) * 8
                ot = opool.tile([128, 272 + 4 * VW], BF16, tag="ot")
                for ai, accx in enumerate((acc0, acc1)):
                    ob = ai * 272
                    sview = accx[:].rearrange("p (j c) -> p j c", c=VW)[
                        :, 0:4, Dh : Dh + 1
                    ]
                    rdst = rc[
                        :, rcol + ai * 4 : rcol + ai * 4 + 4
                    ].rearrange("p (j o) -> p j o", o=1)
                    nc.vector.reciprocal(rdst, sview)
                    if ai == 0:
                        nc.scalar.activation(
                            ot[:, ob : ob + 4 * VW], accx[:, 0 : 4 * VW], Copy
                        )
                    else:
                        nc.vector.tensor_copy(
                            ot[:, ob : ob + 4 * VW], accx[:, 0 : 4 * VW]
                        )
                dgs = []
                for qc in range(8):
                    dg = dgpool.tile([128, 128], BF16, tag="dg")
                    nc.gpsimd.tensor_scalar_mul(
                        dg[:], id_sb[:], rc[:, rcol + qc : rcol + qc + 1]
                    )
                    dgs.append(dg)
                pending.append((h, qt2, ot, dgs))

            # ---- output projection helper ----
            # Deferred (mid-attention) y groups may only use the pa bank:
            # lpa/lpb are the live logits ring there.
            def emit_y(qg, pa_only=False):
                yt = opool.tile([128, 4 * 512], BF16, tag="yt")
                for do in range(4):
                    ypool_, ytag = (pa, "pa") if pa_only else ytags[(qg * 4 + do) % 3]
                    yp = ypool_.tile([128, 512], F32, tag=ytag)
                    for c in range(2):
                        nc.tensor.matmul(
                            yp[:],
                            wo_sb[:, c * D + do * 128 : c * D + do * 128 + 128],
                            ct[:, c * S + qg * 512 : c * S + (qg + 1) * 512],
                            start=(c == 0),
                            stop=(c == 1),
                        )
                    dst = yt[:, do * 512 : (do + 1) * 512]
                    if do % 2 == 0:
                        nc.scalar.activation(dst, yp[:], Copy)
                    else:
                        nc.vector.tensor_copy(dst, yp[:])
                nc.sync.dma_start(
                    yT[:, :, qg * 512 : (qg + 1) * 512].rearrange("c p j -> p c j"),
                    yt[:],
                )

            ytags = [(pa, "pa"), (lp, "lpa"), (lp, "lpb")]
            pending_y = []

            # interleave: proj(tg0,tg1) -> first-iteration kc0-7 can start
            # while proj(tg2,tg3) still runs, warming the exp pipelines.
            # Iterations run qt2-major so y(qg0,qg1) can be emitted as soon
            # as the qt2=0 block (all heads) finishes.
            emit_proj(0)
            emit_proj(1)
            it0 = emit_attn_iter(0, 0)
            for _ in range(8):
                next(it0)
            emit_proj(2)
            for _ in range(4):
                next(it0)
            emit_proj(3)
            for _ in it0:
                pass
            for qt2 in range(NQT):
                for h in range(HL):
                    if h == 0 and qt2 == 0:
                        continue
                    for _ in emit_attn_iter(h, qt2):
                        pass
                if qt2 == 0:
                    pending_y.extend([1, 0])
                else:
                    emit_pending()
                    for qg in (2, 3):
                        emit_y(qg)

    _split_multi_waits(nc)
    return nc


def _attn_v(nc, et_pair, vp, acc0, acc1, h, kc, id_sb):
    eta, etb = et_pair
    # PSUM `start=True` clears has_written for the WHOLE BANK (measured on
    # HW): with several accumulation groups per bank, each group's start
    # would wipe its neighbours' flags and their first contribution would
    # be overwritten. So a single sacrificial matmul per bank carries the
    # one start=True (clearing the bank at kc==0, output discarded), and
    # every real group writes start=False: the first write lands on
    # has_written=0 and overwrites, later ones accumulate. The sacrificial
    # also streams the fresh exp tile via the MOVING operand, which keeps
    # the PE weight-buffer pipeline from loading the et stationaries
    # before the ACT/DVE writes are visible.
    nc.tensor.matmul(
        acc0[:, 4 * VW : 5 * VW],
        id_sb[:],
        eta[:, 0:VW],
        start=(kc == 0),
        stop=(kc == NKC - 1),
    )
    nc.tensor.matmul(
        acc1[:, 4 * VW : 5 * VW],
        id_sb[:],
        etb[:, 0:VW],
        start=(kc == 0),
        stop=(kc == NKC - 1),
    )
    for qc in range(8):
        accx = acc0 if qc < 4 else acc1
        etx = eta if qc < 4 else etb
        j = qc % 4
        nc.tensor.matmul(
            accx[:, j * VW : (j + 1) * VW],
            etx[:, j * 128 : (j + 1) * 128],
            vp[:, (kc * HL + h) * VW : (kc * HL + h + 1) * VW],
            start=False,
            stop=(kc == NKC - 1),
            skip_group_check=True,
        )


_CACHE: dict = {}


def _to_bf16(a):
    import ml_dtypes

    return np.asarray(a, dtype=ml_dtypes.bfloat16)


def _prep_inputs(x, q, Wq, bq, Wk, bk, Wv, bv, Wo, bo):
    x = np.asarray(x, np.float32)
    q = np.asarray(q, np.float32)
    Wq, bq = np.asarray(Wq, np.float32), np.asarray(bq, np.float32)
    Wk, bk = np.asarray(Wk, np.float32), np.asarray(bk, np.float32)
    Wv = np.asarray(Wv, np.float32)
    Wo = np.asarray(Wo, np.float32)

    scale = np.float32(1.0 / np.sqrt(np.float32(Dh)))
    iden = np.eye(128, dtype=np.float32)
    in_maps = []
    for c in range(NCORES):
        b, g = c >> 1, c & 1
        rows = slice(g * DK, (g + 1) * DK)
        xT = np.ascontiguousarray(x[b].T.reshape(4, 128, S))
        qT = np.ascontiguousarray(q[b].T.reshape(4, 128, S))
        wk_h = np.ascontiguousarray(Wk[rows].T.reshape(4, 128, DK))
        wq_h = np.ascontiguousarray((Wq[rows] * scale).T.reshape(4, 128, DK))
        wv_h = np.ascontiguousarray(Wv[rows].T.reshape(4, 128, DK))
        wo_h = np.ascontiguousarray(Wo[:, rows].T.reshape(2, 128, D))
        in_maps.append(
            {
                "xT": _to_bf16(xT),
                "qT": _to_bf16(qT),
                "wk": _to_bf16(wk_h),
                "wq": _to_bf16(wq_h),
                "wv": _to_bf16(wv_h),
                "wo": _to_bf16(wo_h),
                "bk": np.ascontiguousarray(bk[rows].reshape(2, 128, 1)),
                "bq": np.ascontiguousarray(
                    (bq[rows] * scale).reshape(2, 128, 1), dtype=np.float32
                ),
                "iden": _to_bf16(iden),
            }
        )
    return in_maps


def kernel(x, q, Wq, bq, Wk, bk, Wv, bv, Wo, bo):
    _install_drain_patch()
    if "nc" not in _CACHE:
        _CACHE["nc"] = _build()
    nc = _CACHE["nc"]
    in_maps = _prep_inputs(x, q, Wq, bq, Wk, bk, Wv, bv, Wo, bo)
    res = run_bass_kernel_spmd(nc, in_maps, core_ids=list(range(NCORES)))
    # constant term: attention rows sum to exactly 1, so the V bias
    # contributes bv @ Wo.T independent of the data; add with bo on host.
    const = (
        np.asarray(bv, np.float64) @ np.asarray(Wo, np.float64).T
        + np.asarray(bo, np.float64)
    )
    y = np.zeros((B, S, D), np.float64)
    for c in range(NCORES):
        b = c >> 1
        yt = res.results[c]["yT"].astype(np.float64)  # [4, 128, S]
        y[b] += yt.reshape(D, S).T
    y += const
    return y.astype(np.float32)
